# revision 42
# baseline (speedup 1.0000x reference)
"""GCN encoder (2x GCNConv + BatchNorm/ReLU) on 8 Trainium2 NeuronCores.

Math: with s = 1/sqrt(deg+1) (deg = in-degree by dst), the GCN edge norm
factorizes: norm_e = s[src]*s[dst], so for any node features H,
    A(H) := segsum(norm_e * H[src], dst) + H * s^2
          = s * ( segsum( (s*H)[src], dst) + (s*H) )
and GCNConv(H, W, b) = A(H)@W + b = A(H@W) + b, so the whole net needs only
TWO sparse aggregations (layer1 on (s*x)@W1, layer2 on s*post-BN hidden),
and mu / log_std share the second one.

Layer 1 (pull): by linearity A(x@W1) == A(x)@W1, the host-marshaled bf16 x
tables (replicated to every core's DRAM as inputs) ARE the gather tables --
no collective at all.  Slot-aligned gathers + identity-matmul PSUM
accumulation per dst block, then W1 applied post-aggregation.

Layer 2 (push + ReduceScatter; replaces the 253us tab2 AllGather):
  * After BN each core writes its local post-BN slab (s * relu(bn(h1)))
    to local DRAM (tab2loc) -- no collective.
  * Edges are partitioned by SRC core.  Each core gathers its outgoing
    edges' messages from tab2loc in chunks of 128 (dst-sorted), and the PE
    accumulates them FEATURE-MAJOR into per-(dst core, 4-block group) PSUM
    tiles via per-chunk 0/1 assignment matmuls:
        psumT[f, dst] += msg_chunk[e, f]^T  @  M_chunk[e, dst_window]
    (lhsT = the gathered chunk, rhs = a host-built 0/1 matrix).  PE sums
    are deterministic -- dma_scatter_add would race on duplicate dst rows
    on real hardware (verified experimentally).
  * Self-loop term rides along as synthetic (j -> j) edges.
  * Partial slabs go to a [8*128, npc] bf16 accumulator; ONE bf16
    ReduceScatter (priced by OUT size = 1.6MB -> ~57us vs 253us AllGather)
    hands each core its final aggregated feature-major slab.
  * Epilogue: scale by s (column-wise, feature-major), then lhsT IS already
    transposed for the Wcat matmul -- no per-block PE transposes needed.

Static SPMD choreography: chunk counts, matmul windows, and M offsets are
max-over-cores static geometry; all per-core variation lives in tensor
contents (gather idx, M values), with pad positions pointing at row 0 and
zero M columns.

Gather calls carry <=896 idxs (hw SWDGE descriptor ring holds 128 in-flight
entries per engine; larger calls hang the device).  Calls rotate over 4
SWDGE queues; queue_num is re-derived post-compile from the Tile-assigned
DMASW sem lane.
"""

import numpy as np

N_NODES = 50000
N_EDGES = 800000
D_IN = 128
D_HID = 128
D_LAT = 64
BN_EPS = 1e-5
N_CORES = 8
P = 128
LO_CORES = 5  # cores 0..4 form the "lo" table half; 5*6272=31360 < 32768
              # (dma_gather int16 indices address at most 32768 rows per call)

CALL_COLS = 7      # gather call size: 7 cols * 128 = 896 idxs (hw ring cap)
NUM_Q = 4          # SWDGE queues
GB = 4             # dst blocks per PSUM group (layer 2 push)

_CACHE = {}


# ----------------------------------------------------------------------------
# Host-side preprocessing
# ----------------------------------------------------------------------------


def _wrap_idx(lin):
    """dma_gather idx layout: position i -> [i%16, i//16], replicated to 128
    partitions. lin: [n] int array (n % 16 == 0) -> [128, n//16] int16."""
    n = lin.shape[0]
    w = lin.reshape(n // 16, 16).T.astype(np.int16)  # [16, n//16]
    return np.tile(w, (8, 1))


def _pack_calls(D, call_cols):
    """Slice the global column space into calls of <= call_cols columns.

    A call may cover partial blocks; each call carries its piece list
    [(block, col_off_in_call, width, first, last)].
    """
    C0 = np.concatenate([[0], np.cumsum(D)]).astype(np.int64)
    ct = int(C0[-1])
    calls = []
    for c0 in range(0, ct, call_cols):
        c1 = min(c0 + call_cols, ct)
        pieces = []
        for b in range(len(D)):
            lo = max(c0, int(C0[b]))
            hi = min(c1, int(C0[b + 1]))
            if lo < hi:
                pieces.append(
                    (b, lo - c0, hi - lo, lo == int(C0[b]), hi == int(C0[b + 1]))
                )
        calls.append((c0, c1 - c0, tuple(pieces)))
    return C0, calls


def _build_pass(tcoord_src, tkey_dst, n_cores, npc, blocks, call_cols,
                pad_idx, idx_base):
    """Build one gather pass layout (layer-1 pull).

    tcoord_src: per-edge source table coord (already offset for hi pass)
    tkey_dst:   per-edge dst node key in THIS pass's permutation
    Returns D [blocks], C0, calls, idx arrays [n_cores, 128, c_total] int32.
    """
    deg = np.bincount(tkey_dst, minlength=n_cores * npc)
    d3 = deg.reshape(n_cores, blocks, P)
    D = d3.max(axis=(0, 2)).astype(np.int64)
    D = np.maximum(D, 1)
    C0, calls = _pack_calls(D, call_cols)
    c_total = int(C0[-1])

    idx = np.full((n_cores, P, c_total), pad_idx - idx_base, dtype=np.int32)
    eorder = np.argsort(tkey_dst, kind="stable")
    k_s = tkey_dst[eorder]
    src_s = (tcoord_src[eorder] - idx_base).astype(np.int32)
    grp = np.searchsorted(k_s, k_s)
    slot = np.arange(k_s.size) - grp
    core_e = k_s // npc
    local_e = k_s % npc
    b_e = local_e // P
    p_e = local_e % P
    col_e = C0[b_e] + slot
    assert (slot < D[b_e]).all()
    idx[core_e, p_e, col_e] = src_s
    return D, C0, calls, idx, c_total


def _idx_to_wrapped(idx):
    """[n_cores, 128, c_total] int32 -> wrapped int16 [n_cores, 128, 8*c_total].

    Global linear position order is column-major (i = c*128 + p); contiguous
    position chunks map to contiguous wrapped columns, so any call covering
    cols [c0, c1) reads the wrapped slice [:, 8*c0 : 8*c1]."""
    n_cores, _, c_total = idx.shape
    out = np.empty((n_cores, 128, 8 * c_total), dtype=np.int16)
    for k in range(n_cores):
        lin = idx[k].T.reshape(-1)
        out[k] = _wrap_idx(lin)
    return out


def _plan_l2(src, dst, core_of, local_of, l2col_of, node_of, npc, blocks,
             s_l2):
    """Layer-2 push plan: per-src-core edge streams, static chunk/window
    choreography, per-core gather idx + 0/1 M matrices.

    The dst COLUMN order is a separate per-core permutation (l2col_of,
    sorted by total degree) -- balanced across src cores, unlike the main
    dlo-sorted layout, so the max-over-cores chunk envelope stays tight.

    Static geometry (identical across cores, SPMD):
      groups: (kk, g, g0, W, C, m_off, m_cols, windows=((w0, w1, mo), ...))
    Per-core data: idx stream (int, gather rows into tab2loc), M [128, Mtot].
    """
    n_cores = N_CORES
    GR = (blocks + GB - 1) // GB
    group_w = [min(GB * P, npc - g * GB * P) for g in range(GR)]

    # per-core edge lists sorted by (dst_core, dst_l2pos).  Self loops are
    # NOT included here -- they would land only on the own-core stream and
    # blow up the max-over-cores static envelope by ~n_local per dst core;
    # the self term is added in the epilogue from a transposed local gather
    # that runs during the ReduceScatter wait.
    ecore = []
    for k in range(n_cores):
        m = core_of[src] == k
        s_loc = local_of[src[m]]
        d_core = core_of[dst[m]]
        d_loc = l2col_of[dst[m]]
        key = d_core * npc + d_loc
        o = np.argsort(key, kind="stable")
        ecore.append((s_loc[o], d_core[o], d_loc[o], key[o]))

    # group slices per core: searchsorted bounds on key
    # chunk counts per (kk, g): ceil(max_core n / 128)
    groups = []
    per_core_chunks = [[] for _ in range(n_cores)]  # list of (idx128, jrel128)
    m_off = 0
    order_kg = [(kk, g) for kk in range(n_cores) for g in range(GR)]
    for kk, g in order_kg:
        if True:
            g0 = g * GB * P
            W = group_w[g]
            lo_key = kk * npc + g0
            hi_key = kk * npc + g0 + W
            segs = []
            for k in range(n_cores):
                keys = ecore[k][3]
                a = np.searchsorted(keys, lo_key)
                b = np.searchsorted(keys, hi_key)
                segs.append((a, b))
            nmax = max(b - a for a, b in segs)
            C = (nmax + P - 1) // P
            windows = []
            g_m0 = m_off
            for c in range(C):
                w0, w1 = W, 0
                for k in range(n_cores):
                    a, b = segs[k]
                    r0, r1 = a + c * P, min(a + (c + 1) * P, b)
                    if r0 < r1:
                        j = ecore[k][2][r0:r1] - g0
                        w0 = min(w0, int(j.min()))
                        w1 = max(w1, int(j.max()) + 1)
                if w1 <= w0:
                    w0, w1 = 0, 1
                windows.append((w0, w1, m_off - g_m0))
                m_off += w1 - w0
            groups.append((kk, g, g0, W, C, g_m0, m_off - g_m0,
                           tuple(windows)))
            for k in range(n_cores):
                a, b = segs[k]
                for c in range(C):
                    r0, r1 = a + c * P, min(a + (c + 1) * P, b)
                    n = max(0, r1 - r0)
                    idx128 = np.zeros(P, np.int32)
                    jrel = np.full(P, -1, np.int32)
                    if n > 0:
                        sl = ecore[k][0][r0:r1]
                        idx128[:n] = (sl % P) * blocks + sl // P
                        jrel[:n] = ecore[k][2][r0:r1] - g0
                    per_core_chunks[k].append((idx128, jrel))

    total_chunks = sum(gr[4] for gr in groups)
    m_total = m_off
    n_calls = (total_chunks + CALL_COLS - 1) // CALL_COLS

    # per-core tensors
    widx2 = []
    Ms = []
    for k in range(n_cores):
        lin = np.concatenate([c[0] for c in per_core_chunks[k]])
        widx2.append(_wrap_idx(lin))
        M = np.zeros((P, m_total), np.float32)
        t = 0
        for kk, g, g0, W, C, g_m0, g_mc, windows in groups:
            for c in range(C):
                idx128, jrel = per_core_chunks[k][t]
                w0, w1, mo = windows[c]
                rows = np.nonzero(jrel >= 0)[0]
                # M carries the dst-side s factor (out = s * sum), so the
                # epilogue needs no post-matmul scaling at all.
                np.add.at(M, (rows, g_m0 + mo + (jrel[rows] - w0)),
                          s_l2[kk][g0 + jrel[rows]])
                t += 1
        Ms.append(M)

    max_mw = max((gr[6] for gr in groups), default=1)
    max_win = max((w1 - w0 for gr in groups for (w0, w1, _) in gr[7]),
                  default=1)
    return dict(
        groups=tuple(groups),
        total_chunks=total_chunks,
        m_total=m_total,
        n_calls=n_calls,
        max_mw=max_mw,
        max_win=max_win,
        widx2=widx2,
        Ms=Ms,
    )


def _plan(edge_index, n_nodes, n_cores, call_cols):
    src = np.asarray(edge_index[0], dtype=np.int64)
    dst = np.asarray(edge_index[1], dtype=np.int64)

    deg_in = np.bincount(dst, minlength=n_nodes).astype(np.int64)
    s = (1.0 / np.sqrt((deg_in + 1).astype(np.float64))).astype(np.float32)

    n_local = (n_nodes + n_cores - 1) // n_cores
    blocks = (n_local + 1 + P - 1) // P
    npc = blocks * P
    lo_rows = LO_CORES * npc
    assert lo_rows < 32768 and (n_cores * npc - lo_rows) < 32768

    # ---- core assignment: deal by total-degree rank (balances edge load and
    # aligns block-degree profiles across cores).
    order = np.argsort(-deg_in, kind="stable")
    rank_of = np.empty(n_nodes, dtype=np.int64)
    rank_of[order] = np.arange(n_nodes)
    core_of = rank_of % n_cores

    src_is_lo = core_of[src] < LO_CORES
    dlo = np.bincount(dst[src_is_lo], minlength=n_nodes)
    dhi = np.bincount(dst[~src_is_lo], minlength=n_nodes)

    # main layout: per-core locals sorted by lo-degree (tight LO padding)
    local_of = np.empty(n_nodes, dtype=np.int64)
    node2hi = np.empty(n_nodes, dtype=np.int64)
    for k in range(n_cores):
        nodes_k = np.nonzero(core_of == k)[0]
        o = nodes_k[np.argsort(-dlo[nodes_k], kind="stable")]
        local_of[o] = np.arange(o.size)
        o2 = nodes_k[np.argsort(-dhi[nodes_k], kind="stable")]
        node2hi[o2] = k * npc + np.arange(o2.size)
    node2table = core_of * npc + local_of

    # ---- gather-source row numbering: tables are [(core,part), (block,feat)]
    # 2-D tensors, so node (core k, local j=b*128+p) lives at flat row
    # (k*128+p)*blocks + b of its half (hi half: k-LO_CORES).
    def kpb_row(core, local, core0):
        return ((core - core0) * P + local % P) * blocks + local // P

    node2row = np.where(
        core_of < LO_CORES,
        kpb_row(core_of, local_of, 0),
        kpb_row(core_of, local_of, LO_CORES),
    )
    pad_lo = kpb_row(0, npc - 1, 0)
    pad_hi = kpb_row(N_CORES - 1, npc - 1, LO_CORES)

    # ---- LO pass on the main permutation
    D_lo, C0_lo, calls_lo, idx_lo, ct_lo = _build_pass(
        node2row[src[src_is_lo]], node2table[dst[src_is_lo]], n_cores, npc,
        blocks, call_cols, pad_lo, 0,
    )

    # ---- HI pass on the hi permutation
    D_hi, C0_hi, calls_hi, idx_hi, ct_hi = _build_pass(
        node2row[src[~src_is_lo]], node2hi[dst[~src_is_lo]], n_cores, npc,
        blocks, call_cols, pad_hi, 0,
    )

    # ---- combine map: main-layout local j gets acc_hi[himap[j]] added
    himap = np.full((n_cores, npc), npc - 1, dtype=np.int64)  # pad -> pad row
    for k in range(n_cores):
        nodes_k = np.nonzero(core_of == k)[0]
        himap[k, local_of[nodes_k]] = node2hi[nodes_k] % npc

    widx_lo = _idx_to_wrapped(idx_lo)
    widx_hi = _idx_to_wrapped(idx_hi)
    himap_row = (himap % P) * blocks + himap // P
    widx_cb = np.stack([_wrap_idx(himap_row[k]) for k in range(n_cores)])

    # per-core node lists and s in the MAIN layout
    node_of = np.full((n_cores, npc), -1, dtype=np.int64)
    s_arr = np.zeros((n_cores, P, blocks), dtype=np.float32)
    for k in range(n_cores):
        nodes_k = np.nonzero(core_of == k)[0]
        loc = local_of[nodes_k]
        node_of[k, loc] = nodes_k
        s_arr[k, loc % P, loc // P] = s[nodes_k]

    # layer-2 dst column permutation: per-core total-degree sort (balanced
    # per-src-core edge counts -> tight static chunk envelope)
    l2col_of = np.empty(n_nodes, dtype=np.int64)
    node_of_l2 = np.full((n_cores, npc), -1, dtype=np.int64)
    s_l2 = np.zeros((n_cores, npc), dtype=np.float32)
    wself = []
    for k in range(n_cores):
        nodes_k = np.nonzero(core_of == k)[0]
        o = nodes_k[np.argsort(-deg_in[nodes_k], kind="stable")]
        l2col_of[o] = np.arange(o.size)
        node_of_l2[k, : o.size] = o
        s_l2[k, : o.size] = s[o]
        rows = np.zeros(npc, dtype=np.int64)
        loc = local_of[o]
        rows[: o.size] = (loc % P) * blocks + loc // P
        wself.append(_wrap_idx(rows))

    l2 = _plan_l2(src, dst, core_of, local_of, l2col_of, node_of, npc, blocks,
                  s_l2)

    return dict(
        s=s,
        node2table=node2table,
        node_of=node_of,
        npc=npc,
        blocks=blocks,
        n_local=n_local,
        lo_rows=lo_rows,
        s_arr=s_arr,
        D_lo=D_lo, C0_lo=C0_lo, calls_lo=calls_lo, idx_lo=idx_lo, ct_lo=ct_lo,
        D_hi=D_hi, C0_hi=C0_hi, calls_hi=calls_hi, idx_hi=idx_hi, ct_hi=ct_hi,
        himap=himap,
        widx_lo=widx_lo, widx_hi=widx_hi, widx_cb=widx_cb,
        node_of_l2=node_of_l2, s_l2=s_l2, wself=wself,
        l2=l2,
    )


def _to_bf16(a):
    import ml_dtypes

    return a.astype(ml_dtypes.bfloat16)


def _host_inputs(plan, x, W1, Wmu, Wls, bmu, bls, gamma, beta):
    npc = plan["npc"]
    node_of = plan["node_of"]
    s = plan["s"]
    l2 = plan["l2"]
    wcat = np.concatenate([Wmu, Wls], axis=1).astype(np.float32)
    bcat = np.concatenate([bmu, bls]).astype(np.float32).reshape(D_HID, 1)

    # s-prescaled x in the partition-major table layout:
    # row (core*128 + p), col (block*128 + f) holds node (core, b*128+p).
    blocks = npc // P
    xs_tab = np.zeros((N_CORES * npc, x.shape[1]), dtype=np.float32)
    for k in range(N_CORES):
        nodes = node_of[k]
        valid = nodes >= 0
        xs_tab[k * npc + np.nonzero(valid)[0]] = (
            x[nodes[valid]] * s[nodes[valid]][:, None]
        )
    xs_kpb = (
        xs_tab.reshape(N_CORES, blocks, P, D_IN)
        .transpose(0, 2, 1, 3)
        .reshape(N_CORES * P, blocks * D_IN)
    )
    xs_kpb = _to_bf16(xs_kpb)
    xs_lo = np.ascontiguousarray(xs_kpb[: LO_CORES * P])
    xs_hi = np.ascontiguousarray(xs_kpb[LO_CORES * P :])

    per_core = []
    for k in range(N_CORES):
        sa = plan["s_arr"][k]  # [P, blocks]
        # layer-2 epilogue scale: s per output partition (L2 column order:
        # block b, partition p <-> l2 position b*128+p)
        scol2 = np.ascontiguousarray(
            plan["s_l2"][k].reshape(blocks, P).T.astype(np.float32))
        per_core.append(
            {
                "xslo": xs_lo,
                "xshi": xs_hi,
                "xso": np.ascontiguousarray(xs_kpb[k * P : (k + 1) * P]),
                "W1": np.ascontiguousarray(_to_bf16(W1)),
                "Wcat": np.ascontiguousarray(_to_bf16(wcat)),
                "bcat": bcat,
                "s_arr": np.ascontiguousarray(sa),
                "widx_lo": np.ascontiguousarray(plan["widx_lo"][k]),
                "widx_hi": np.ascontiguousarray(plan["widx_hi"][k]),
                "widx_cb": np.ascontiguousarray(plan["widx_cb"][k]),
                "gamma": gamma.astype(np.float32).reshape(D_HID, 1),
                "beta": beta.astype(np.float32).reshape(D_HID, 1),
                "widx2": np.ascontiguousarray(l2["widx2"][k]),
                "M2": np.ascontiguousarray(_to_bf16(l2["Ms"][k])),
                "scol2": scol2,
                "wself": np.ascontiguousarray(plan["wself"][k]),
            }
        )
    return per_core


def _postprocess(plan, outs):
    n_nodes = int(plan["node_of"].max()) + 1
    mu = np.zeros((n_nodes, D_LAT), dtype=np.float32)
    ls = np.zeros((n_nodes, D_LAT), dtype=np.float32)
    node_of = plan["node_of_l2"]  # out_cat columns are in the L2 permutation
    npc = node_of.shape[1]
    blocks = npc // 128
    for k in range(N_CORES):
        nodes = node_of[k]
        valid = nodes >= 0
        o = np.asarray(outs[k]).astype(np.float32).reshape(128, blocks, 128)
        o = o.transpose(1, 0, 2).reshape(npc, 128)  # node-major (b*128+p)
        mu[nodes[valid]] = o[valid.nonzero()[0], :D_LAT]
        ls[nodes[valid]] = o[valid.nonzero()[0], D_LAT:]
    return mu, ls


# ----------------------------------------------------------------------------
# Device program
# ----------------------------------------------------------------------------


def _build_program(geom):
    from concourse import bacc, bass, mybir, tile
    from concourse.masks import make_identity

    (npc, blocks, D_lo, calls_lo, ct_lo, D_hi, calls_hi, ct_hi, n_real,
     call_cols, lo_rows, l2g) = geom
    (groups, total_chunks, m_total, n_calls, max_mw, max_win) = l2g
    D_lo, D_hi = list(D_lo), list(D_hi)
    f32 = mybir.dt.float32
    bf16 = mybir.dt.bfloat16
    i16 = mybir.dt.int16
    GR = (blocks + GB - 1) // GB

    nc = bacc.Bacc("TRN2", target_bir_lowering=False, debug=False,
                   num_devices=N_CORES, num_swdge_queues=NUM_Q)

    t_xslo = nc.dram_tensor("xslo", [LO_CORES * P, npc], bf16,
                            kind="ExternalInput")
    t_xshi = nc.dram_tensor("xshi", [(N_CORES - LO_CORES) * P, npc], bf16,
                            kind="ExternalInput")
    t_xso = nc.dram_tensor("xso", [P, npc], bf16, kind="ExternalInput")
    t_W1 = nc.dram_tensor("W1", [P, D_HID], bf16, kind="ExternalInput")
    t_Wcat = nc.dram_tensor("Wcat", [D_HID, P], bf16, kind="ExternalInput")
    t_bcat = nc.dram_tensor("bcat", [P, 1], f32, kind="ExternalInput")
    t_sarr = nc.dram_tensor("s_arr", [P, blocks], f32, kind="ExternalInput")
    t_wlo = nc.dram_tensor("widx_lo", [P, 8 * ct_lo], i16, kind="ExternalInput")
    t_whi = nc.dram_tensor("widx_hi", [P, 8 * ct_hi], i16, kind="ExternalInput")
    t_wcb = nc.dram_tensor("widx_cb", [P, npc // 16], i16, kind="ExternalInput")
    t_gamma = nc.dram_tensor("gamma", [D_HID, 1], f32, kind="ExternalInput")
    t_beta = nc.dram_tensor("beta", [D_HID, 1], f32, kind="ExternalInput")
    t_widx2 = nc.dram_tensor("widx2", [P, total_chunks * P // 16], i16,
                             kind="ExternalInput")
    t_M2 = nc.dram_tensor("M2", [P, m_total], bf16, kind="ExternalInput")
    t_scol2 = nc.dram_tensor("scol2", [P, blocks], f32, kind="ExternalInput")
    t_wself = nc.dram_tensor("wself", [P, npc // 16], i16,
                             kind="ExternalInput")
    t_out = nc.dram_tensor("out_cat", [P, npc], bf16, kind="ExternalOutput")

    tab2loc = nc.dram_tensor("tab2loc", [npc, P], bf16)
    acc1 = nc.dram_tensor("acc_hi1", [P, npc], bf16)
    accL2 = nc.dram_tensor("accL2", [N_CORES * P, npc], bf16)
    rs_out = nc.dram_tensor("rs_out", [P, npc], bf16)
    st_in = nc.dram_tensor("st_in", [2 * D_HID], f32)
    st_out = nc.dram_tensor("st_out", [N_CORES * 2 * D_HID], f32,
                            addr_space="Shared")

    groups_rep = [list(range(N_CORES))]
    inv_n = 1.0 / float(n_real)

    with tile.TileContext(nc) as tc:
        with (
            tc.tile_pool(name="persist", bufs=1) as persist,
            tc.tile_pool(name="stream", bufs=4) as stream,
            tc.tile_pool(name="gath", bufs=3) as gath,
            tc.tile_pool(name="gath2", bufs=4) as gath2,
            tc.tile_pool(name="mpool", bufs=3) as mpool,
            tc.tile_pool(name="ppool", bufs=3) as ppool,
            tc.tile_pool(name="gathcb", bufs=1) as gathcb,
            tc.tile_pool(name="small", bufs=1) as small,
            tc.tile_pool(name="ps", bufs=2, space="PSUM") as psp,
            tc.tile_pool(name="ps_blk", bufs=3, space="PSUM") as psblk,
            tc.tile_pool(name="ps_acc", bufs=1, space="PSUM") as psacc,
        ):
            H = persist.tile([P, npc], bf16, tag="H")
            ownslab = persist.tile([P, npc], bf16, tag="ownslab")
            accsb = persist.tile([P, npc], bf16, tag="accsb")
            outsb = persist.tile([P, npc], bf16, tag="outsb")
            wlo = persist.tile([P, 8 * ct_lo], i16, tag="wlo")
            whi = persist.tile([P, 8 * ct_hi], i16, tag="whi")
            wcb = persist.tile([P, npc // 16], i16, tag="wcb")
            widx2 = persist.tile([P, total_chunks * P // 16], i16,
                                 tag="widx2")
            wself = persist.tile([P, npc // 16], i16, tag="wself")
            selfT = persist.tile([P, npc], bf16, tag="selfT")
            w1 = small.tile([P, D_HID], bf16, tag="w1")
            wcat = small.tile([D_HID, P], bf16, tag="wcat")
            sarr = small.tile([P, blocks], f32, tag="sarr")
            scol2 = small.tile([P, blocks], f32, tag="scol2")
            gcol = small.tile([D_HID, 1], f32, tag="gcol")
            bcol = small.tile([D_HID, 1], f32, tag="bcol")
            bccol = small.tile([P, 1], f32, tag="bccol")
            ident = small.tile([P, P], bf16, tag="ident")
            identf = small.tile([P, P], f32, tag="identf")
            zeroM = small.tile([P, GB * P], bf16, tag="zeroM")
            ones_col = small.tile([P, 1], bf16, tag="ones_col")
            ones_row = small.tile([1, P], f32, tag="ones_row")
            grep = small.tile([P, P], f32, tag="grep")
            brep = small.tile([P, P], f32, tag="brep")
            grep4 = small.tile([P, 4 * P], bf16, tag="grep4")
            brep4 = small.tile([P, 4 * P], bf16, tag="brep4")
            bcrep = small.tile([P, P], f32, tag="bcrep")

            nc.sync.dma_start(out=whi[:], in_=t_whi[:])
            nc.sync.dma_start(out=wlo[:], in_=t_wlo[:])
            nc.sync.dma_start(out=wcb[:], in_=t_wcb[:])
            nc.sync.dma_start(out=widx2[:], in_=t_widx2[:])
            nc.sync.dma_start(out=wself[:], in_=t_wself[:])
            nc.sync.dma_start(out=scol2[:], in_=t_scol2[:])
            nc.sync.dma_start(out=w1[:], in_=t_W1[:])
            nc.sync.dma_start(out=wcat[:], in_=t_Wcat[:])
            nc.sync.dma_start(out=sarr[:], in_=t_sarr[:])
            nc.sync.dma_start(out=gcol[:], in_=t_gamma[:])
            nc.sync.dma_start(out=bcol[:], in_=t_beta[:])
            nc.sync.dma_start(out=bccol[:], in_=t_bcat[:])
            make_identity(nc, ident[:])
            make_identity(nc, identf[:])
            nc.vector.memset(zeroM[:], 0.0)
            nc.vector.memset(ones_col[:], 1.0)
            nc.vector.memset(ones_row[:], 1.0)
            warm = small.tile([1, 1], f32, tag="warm")
            nc.vector.memset(warm[:], 1.0)
            nc.scalar.activation(out=warm[:], in_=warm[:],
                                 func=mybir.ActivationFunctionType.Sqrt)

            def outer_bcast(col_ap, dst_tile):
                pst = psp.tile([P, P], f32, space="PSUM", tag="ps_big")
                nc.tensor.transpose(out=pst[0:1, :], in_=col_ap,
                                    identity=identf[:])
                row = stream.tile([1, P], f32, tag="rowbuf")
                nc.vector.tensor_copy(out=row[:], in_=pst[0:1, :])
                psb = psp.tile([P, P], f32, space="PSUM", tag="ps_big")
                nc.tensor.matmul(out=psb[:], lhsT=ones_row[:], rhs=row[:],
                                 start=True, stop=True)
                nc.vector.tensor_copy(out=dst_tile[:], in_=psb[:])

            outer_bcast(bccol[:], bcrep)

            # --- layer 1 gathers raw (s*x): A(x@W1) == A(x)@W1, so W1 is
            # applied per-block AFTER aggregation; the host-marshaled xslo/
            # xshi inputs ARE the gather tables (no stage-1, no table write).
            nc.sync.dma_start(out=ownslab[:], in_=t_xso[:, :])

            ps_stats = psacc.tile([D_HID, 2], f32, space="PSUM",
                                  tag="ps_stats")

            qn = [0]

            def next_q():
                q = qn[0]
                qn[0] = (qn[0] + 1) % NUM_Q
                return q

            def gather(out_ap, table_ap, widx_ap, n_idx):
                nc.gpsimd.dma_gather(
                    out_ap.rearrange("p (c f) -> p c f", f=P),
                    table_ap,
                    widx_ap,
                    num_idxs=n_idx,
                    num_idxs_reg=n_idx,
                    elem_size=P,
                    queue_num=next_q(),
                )

            def aggregate(tlo, thi, acc_t, selfslab):
                """Layer-1 pull aggregation -> H blocks + BN stats."""
                grp = {}

                def blk_view(b, pre):
                    g, r = b // GB, b % GB
                    if g not in grp:
                        grp[g] = psblk.tile([P, GB * P], f32, space="PSUM",
                                            tag="ps_blk",
                                            name=f"{pre}_{g}")
                    return grp[g][:, r * P : (r + 1) * P]

                g_open = set()

                def g_last(b):
                    return min((b // GB) * GB + GB - 1, blocks - 1)

                def flags(b, is_first_mm, is_last_mm):
                    g = b // GB
                    start = is_first_mm and g not in g_open
                    if start:
                        g_open.add(g)
                    stop = is_last_mm and b == g_last(b)
                    return start, stop

                for c0, cols, pieces in calls_hi:
                    gt = gath.tile([P, call_cols * P], bf16, tag="gt")
                    gather(gt[:, : cols * P],
                           thi.rearrange("q (r f) -> (q r) f", f=P),
                           whi[:, 8 * c0 : 8 * (c0 + cols)], cols * P)
                    for b, o, d, first, last in pieces:
                        bv = blk_view(b, "psg_hi")
                        for i in range(d):
                            st_, sp_ = flags(b, first and i == 0,
                                             last and i == d - 1)
                            nc.tensor.matmul(
                                out=bv, lhsT=ident[:],
                                rhs=gt[:, (o + i) * P : (o + i + 1) * P],
                                start=st_, stop=sp_)
                        if last and b == g_last(b):
                            g = b // GB
                            w = (b % GB) + 1
                            nc.vector.tensor_copy(
                                out=accsb[:, g * GB * P : g * GB * P + w * P],
                                in_=grp.pop(g)[:, : w * P])
                nc.sync.dma_start(out=acc_t[:, :], in_=accsb[:])

                # combine gathers (cbt[:, j] = acc_t[himap[j]]) are issued
                # a few calls INTO the lo stream so the acc-write latency
                # hides behind lo gather work.
                cbt = gathcb.tile([P, npc], bf16, tag="gtcb")

                grp.clear()
                g_open.clear()

                def start_block(b):
                    bv = blk_view(b, "psg_lo")
                    st_, _ = flags(b, True, False)
                    nc.tensor.matmul(out=bv, lhsT=ident[:],
                                     rhs=selfslab[:, b * P : (b + 1) * P],
                                     start=st_, stop=False)
                    return bv

                cb_at = min(1, (D_lo[0] - 1) // call_cols)
                for ci, (c0, cols, pieces) in enumerate(calls_lo):
                    if ci == cb_at:
                        for cb0 in range(0, blocks, call_cols):
                            cb1 = min(cb0 + call_cols, blocks)
                            gather(cbt[:, cb0 * P : cb1 * P],
                                   acc_t.rearrange("q (r f) -> (q r) f", f=P),
                                   wcb[:, 8 * cb0 : 8 * cb1],
                                   (cb1 - cb0) * P)
                    gt = gath.tile([P, call_cols * P], bf16, tag="gt")
                    gather(gt[:, : cols * P],
                           tlo.rearrange("q (r f) -> (q r) f", f=P),
                           wlo[:, 8 * c0 : 8 * (c0 + cols)], cols * P)
                    for b, o, d, first, last in pieces:
                        if first:
                            start_block(b)
                        bv = blk_view(b, "psg_lo")
                        for i in range(d):
                            nc.tensor.matmul(
                                out=bv, lhsT=ident[:],
                                rhs=gt[:, (o + i) * P : (o + i + 1) * P],
                                start=False, stop=False)
                        if last:
                            _, sp_ = flags(b, False, True)
                            nc.tensor.matmul(
                                out=bv, lhsT=ident[:],
                                rhs=cbt[:, b * P : (b + 1) * P],
                                start=False, stop=sp_)
                        if last and b == g_last(b):
                            g = b // GB
                            gt_ps = grp.pop(g)
                            for r in range(b % GB + 1):
                                bb = g * GB + r
                                sl = slice(bb * P, (bb + 1) * P)
                                # U = s*(agg); h1 = U @ W1
                                ub = stream.tile([P, P], bf16, tag="ub")
                                nc.vector.tensor_scalar_mul(
                                    ub[:], gt_ps[:, r * P : (r + 1) * P],
                                    sarr[:, bb : bb + 1])
                                psT = psp.tile([P, P], bf16, space="PSUM",
                                               tag="ps_bigT")
                                nc.tensor.transpose(out=psT[:], in_=ub[:],
                                                    identity=ident[:])
                                uT = stream.tile([P, P], bf16, tag="uT")
                                nc.vector.tensor_copy(out=uT[:], in_=psT[:])
                                ps1 = psp.tile([P, P], f32, space="PSUM",
                                               tag="ps_big")
                                nc.tensor.matmul(out=ps1[:], lhsT=uT[:],
                                                 rhs=w1[:], start=True,
                                                 stop=True)
                                nc.vector.tensor_copy(out=H[:, sl],
                                                      in_=ps1[:])
                                sq = stream.tile([P, D_HID], bf16,
                                                 tag="sq")
                                nc.scalar.square(out=sq[:], in_=H[:, sl])
                                nc.tensor.matmul(
                                    out=ps_stats[:, 0:1], lhsT=H[:, sl],
                                    rhs=ones_col[:],
                                    start=(bb == 0), stop=False)
                                nc.tensor.matmul(
                                    out=ps_stats[:, 1:2], lhsT=sq[:],
                                    rhs=ones_col[:],
                                    start=False,
                                    stop=(bb == blocks - 1))

            aggregate(t_xslo, t_xshi, acc1, ownslab)

            # --- BN ---------------------------------------------------------
            st = small.tile([D_HID, 2], f32, tag="st")
            nc.vector.tensor_copy(out=st[:], in_=ps_stats[:])
            nc.sync.dma_start(out=st_in[:], in_=st[:])
            nc.gpsimd.collective_compute(
                "AllGather", mybir.AluOpType.bypass,
                replica_groups=groups_rep,
                ins=[st_in[:]], outs=[st_out[:]],
            )
            st8 = small.tile([D_HID, 2 * N_CORES], f32, tag="st8")
            nc.sync.dma_start(
                out=st8.rearrange("p (c t) -> p c t", t=2),
                in_=st_out[:].rearrange("(c p t) -> p c t", p=D_HID, t=2),
            )
            st2 = small.tile([D_HID, 2], f32, tag="st2")
            nc.vector.reduce_sum(
                out=st2[:],
                in_=st8.rearrange("p (c t) -> p t c", t=2),
                axis=mybir.AxisListType.X)

            eps_col = small.tile([D_HID, 1], f32, tag="eps_col")
            nc.vector.memset(eps_col[:], BN_EPS)
            mean = small.tile([D_HID, 1], f32, tag="mean")
            msq = small.tile([D_HID, 1], f32, tag="msq")
            var = small.tile([D_HID, 1], f32, tag="var")
            std = small.tile([D_HID, 1], f32, tag="std")
            istd = small.tile([D_HID, 1], f32, tag="istd")
            gp = small.tile([D_HID, 1], f32, tag="gp")
            bp_ = small.tile([D_HID, 1], f32, tag="bp")
            nc.vector.tensor_scalar_mul(mean[:], st2[:, 0:1], inv_n)
            nc.vector.tensor_scalar_mul(msq[:], st2[:, 1:2], inv_n)
            nc.scalar.square(out=var[:], in_=mean[:])
            nc.vector.tensor_tensor(out=var[:], in0=msq[:], in1=var[:],
                                    op=mybir.AluOpType.subtract)
            nc.scalar.activation(out=std[:], in_=var[:],
                                 func=mybir.ActivationFunctionType.Sqrt,
                                 bias=eps_col[:])
            nc.vector.reciprocal(out=istd[:], in_=std[:])
            nc.vector.tensor_tensor(out=gp[:], in0=gcol[:], in1=istd[:],
                                    op=mybir.AluOpType.mult)
            nc.vector.tensor_tensor(out=bp_[:], in0=mean[:], in1=gp[:],
                                    op=mybir.AluOpType.mult)
            nc.vector.tensor_tensor(out=bp_[:], in0=bcol[:], in1=bp_[:],
                                    op=mybir.AluOpType.subtract)
            outer_bcast(gp[:], grep)
            outer_bcast(bp_[:], brep)
            for r in range(4):
                nc.vector.tensor_copy(out=grep4[:, r * P : (r + 1) * P],
                                      in_=grep[:])
                nc.vector.tensor_copy(out=brep4[:, r * P : (r + 1) * P],
                                      in_=brep[:])

            # BN affine + ReLU + s-scale, written to local DRAM table
            # (tab2loc rows = p*blocks + b -- the layer-2 gather source).
            t2v = tab2loc[:, :].rearrange("(p b) f -> p b f", b=blocks)
            for b0 in range(0, blocks, 4):
                b1 = min(b0 + 4, blocks)
                w = (b1 - b0) * P
                t1 = stream.tile([P, 4 * P], bf16, tag="bn1")
                nc.vector.tensor_tensor(out=t1[:, :w], in0=H[:, b0 * P:b0 * P + w],
                                        in1=grep4[:, :w],
                                        op=mybir.AluOpType.mult)
                nc.vector.tensor_tensor(out=t1[:, :w], in0=t1[:, :w],
                                        in1=brep4[:, :w],
                                        op=mybir.AluOpType.add)
                nc.scalar.activation(out=t1[:, :w], in_=t1[:, :w],
                                     func=mybir.ActivationFunctionType.Relu)
                t2 = stream.tile([P, 4 * P], bf16, tag="bn2")
                for b in range(b0, b1):
                    r = b - b0
                    nc.vector.tensor_scalar_mul(
                        t2[:, r * P : (r + 1) * P],
                        t1[:, r * P : (r + 1) * P], sarr[:, b : b + 1])
                nc.sync.dma_start(out=t2v[:, b0:b1, :],
                                  in_=t2[:, :w].rearrange(
                                      "p (b f) -> p b f", f=P))

            # --- layer 2: push aggregation (feature-major M-matmuls) -------
            chunk_t = 0
            issued = {}

            def ensure_call(t):
                ci = t // CALL_COLS
                if ci not in issued:
                    t0 = ci * CALL_COLS
                    cols = min(CALL_COLS, total_chunks - t0)
                    gt = gath2.tile([P, CALL_COLS * P], bf16, tag="gt2")
                    gather(gt[:, : cols * P], tab2loc[:, :],
                           widx2[:, 8 * t0 : 8 * (t0 + cols)], cols * P)
                    issued[ci] = gt
                return issued[ci], t % CALL_COLS

            def emit_self():
                # self term: gather own rows in L2 column order, transpose
                # to feature-major.  Runs during the RS#1 transfer.
                for c0 in range(0, blocks, CALL_COLS):
                    c1 = min(c0 + CALL_COLS, blocks)
                    sg = stream.tile([P, CALL_COLS * P], bf16, tag="sg")
                    gather(sg[:, : (c1 - c0) * P], tab2loc[:, :],
                           wself[:, 8 * c0 : 8 * c1], (c1 - c0) * P)
                    for b in range(c0, c1):
                        psT2 = psp.tile([P, P], bf16, space="PSUM",
                                        tag="ps_bigT")
                        nc.tensor.transpose(
                            out=psT2[:],
                            in_=sg[:, (b - c0) * P : (b - c0 + 1) * P],
                            identity=ident[:])
                        nc.vector.tensor_copy(
                            out=selfT[:, b * P : (b + 1) * P], in_=psT2[:])

            for gi, (kk, g, g0, W, C, g_m0, g_mc, windows) in enumerate(groups):
                Mt = None
                if g_mc > 0:
                    Mt = mpool.tile([P, max_mw], bf16, tag="mt")
                    nc.scalar.dma_start(out=Mt[:, :g_mc],
                                        in_=t_M2[:, g_m0 : g_m0 + g_mc])
                ps2 = psblk.tile([P, GB * P], f32, space="PSUM", tag="ps_blk",
                                 name=f"ps2_{gi}")
                nc.tensor.matmul(out=ps2[:, :W], lhsT=ident[:],
                                 rhs=zeroM[:, :W], start=True, stop=(C == 0))
                for c in range(C):
                    gt, pos = ensure_call(chunk_t)
                    w0, w1, mo = windows[c]
                    nc.tensor.matmul(
                        out=ps2[:, w0:w1],
                        lhsT=gt[:, pos * P : (pos + 1) * P],
                        rhs=Mt[:, mo : mo + (w1 - w0)],
                        start=False, stop=(c == C - 1))
                    chunk_t += 1
                pt = ppool.tile([P, GB * P], bf16, tag="pt")
                nc.vector.tensor_copy(out=pt[:, :W], in_=ps2[:, :W])
                nc.sync.dma_start(out=accL2[kk * P : (kk + 1) * P,
                                            g0 : g0 + W],
                                  in_=pt[:, :W])
            emit_self()

            nc.gpsimd.collective_compute(
                "ReduceScatter", mybir.AluOpType.add,
                replica_groups=groups_rep,
                ins=[accL2[:]], outs=[rs_out[:]],
            )

            # --- epilogue: Wcat matmuls (feature-major lhsT, no transposes;
            # self term folded as a second accumulating matmul; s applied
            # per-partition AFTER the matmul on the Act engine).
            # Piece A is emitted first so it overlaps the RS#2 transfer.
            def epilogue(rs_t, b0, b1, off):
                for c0 in range(b0, b1, GB):
                    c1 = min(c0 + GB, b1)
                    w = (c1 - c0) * P
                    nc.sync.dma_start(
                        out=H[:, c0 * P : c0 * P + w],
                        in_=rs_t[:, c0 * P - off : c0 * P - off + w])
                    for b in range(c0, c1):
                        pso = psp.tile([P, P], f32, space="PSUM",
                                       tag="ps_big")
                        nc.tensor.matmul(out=pso[:],
                                         lhsT=H[:, b * P : (b + 1) * P],
                                         rhs=wcat[:], start=True, stop=False)
                        nc.tensor.matmul(out=pso[:],
                                         lhsT=selfT[:, b * P : (b + 1) * P],
                                         rhs=wcat[:], start=False, stop=True)
                        su = stream.tile([P, P], f32, tag="su")
                        nc.scalar.activation(
                            out=su[:], in_=pso[:],
                            func=mybir.ActivationFunctionType.Copy,
                            scale=scol2[:, b : b + 1])
                        nc.vector.tensor_add(
                            out=outsb[:, b * P : (b + 1) * P], in0=su[:],
                            in1=bcrep[:])
                    nc.sync.dma_start(out=t_out[:, c0 * P : c0 * P + w],
                                      in_=outsb[:, c0 * P : c0 * P + w])

            epilogue(rs_out, 0, blocks, 0)

    nc.compile()
    if NUM_Q > 1:
        # Tile assigns SWDGE completion-sem lanes (DMASW0..7) round-robin in
        # SCHEDULED order, which differs from creation order.  Each sem lane
        # must be driven by a single SWDGE queue, so re-derive queue_num from
        # the assigned lane.
        import re as _re

        for _blk in nc.m.functions[0].blocks:
            for _inst in _blk.instructions:
                if type(_inst).__name__ == "InstDMAGatherAnt":
                    _m = _re.search(r"DMASW(\d+)_",
                                    str(_inst.sync_info.on_update[0]))
                    _inst.queue_num = int(_m.group(1)) % NUM_Q
    return nc


# ----------------------------------------------------------------------------
# Entry point
# ----------------------------------------------------------------------------

_IN_NAMES = ["xslo", "xshi", "xso", "W1", "Wcat", "bcat", "s_arr",
             "widx_lo", "widx_hi", "widx_cb", "gamma", "beta",
             "widx2", "M2", "scol2", "wself"]


def _geom(plan, call_cols):
    l2 = plan["l2"]
    l2g = (
        tuple(l2["groups"]),
        int(l2["total_chunks"]),
        int(l2["m_total"]),
        int(l2["n_calls"]),
        int(l2["max_mw"]),
        int(l2["max_win"]),
    )
    return (
        plan["npc"],
        plan["blocks"],
        tuple(int(d) for d in plan["D_lo"]),
        tuple(plan["calls_lo"]),
        plan["ct_lo"],
        tuple(int(d) for d in plan["D_hi"]),
        tuple(plan["calls_hi"]),
        plan["ct_hi"],
        int(plan["node_of"].max()) + 1,
        call_cols,
        plan["lo_rows"],
        l2g,
    )


def _run_hw(nc, per_core, trace=False, trace_cores=None):
    from concourse import bass_utils

    in_maps = [{nm: per_core[k][nm] for nm in _IN_NAMES} for k in range(N_CORES)]
    res = bass_utils.run_bass_kernel_spmd(
        nc, in_maps, core_ids=list(range(N_CORES)), trace=trace,
        trace_cores=trace_cores,
    )
    outs = [res.results[k]["out_cat"] for k in range(N_CORES)]
    return outs, res


def kernel(x, edge_index, W1, b1, gamma, beta, Wmu, bmu, Wls, bls):
    x = np.asarray(x, dtype=np.float32)
    edge_index = np.asarray(edge_index)
    W1 = np.asarray(W1, dtype=np.float32)
    gamma = np.asarray(gamma, dtype=np.float32)
    beta = np.asarray(beta, dtype=np.float32)
    Wmu = np.asarray(Wmu, dtype=np.float32)
    bmu = np.asarray(bmu, dtype=np.float32)
    Wls = np.asarray(Wls, dtype=np.float32)
    bls = np.asarray(bls, dtype=np.float32)

    plan = _plan(edge_index, x.shape[0], N_CORES, call_cols=CALL_COLS)
    per_core = _host_inputs(plan, x, W1, Wmu, Wls, bmu, bls, gamma, beta)

    geom = _geom(plan, CALL_COLS)
    if geom not in _CACHE:
        _CACHE[geom] = _build_program(geom)
    nc = _CACHE[geom]

    outs, _ = _run_hw(nc, per_core, trace=False)
    mu, ls = _postprocess(plan, outs)
    return mu, ls


# "M2" key is provided by _host_inputs; keep name mapping for clarity.


# revision 46
# speedup vs baseline: 1.0086x; 1.0086x over previous
"""GCN encoder (2x GCNConv + BatchNorm/ReLU) on 8 Trainium2 NeuronCores.

Math: with s = 1/sqrt(deg+1) (deg = in-degree by dst), the GCN edge norm
factorizes: norm_e = s[src]*s[dst], so for any node features H,
    A(H) := segsum(norm_e * H[src], dst) + H * s^2
          = s * ( segsum( (s*H)[src], dst) + (s*H) )
and GCNConv(H, W, b) = A(H)@W + b = A(H@W) + b, so the whole net needs only
TWO sparse aggregations (layer1 on (s*x)@W1, layer2 on s*post-BN hidden),
and mu / log_std share the second one.

Layer 1 (pull): by linearity A(x@W1) == A(x)@W1, the host-marshaled bf16 x
tables (replicated to every core's DRAM as inputs) ARE the gather tables --
no collective at all.  Slot-aligned gathers + identity-matmul PSUM
accumulation per dst block, then W1 applied post-aggregation.

Layer 2 (push + ReduceScatter; replaces the 253us tab2 AllGather):
  * After BN each core writes its local post-BN slab (s * relu(bn(h1)))
    to local DRAM (tab2loc) -- no collective.
  * Edges are partitioned by SRC core.  Each core gathers its outgoing
    edges' messages from tab2loc in chunks of 128 (dst-sorted), and the PE
    accumulates them FEATURE-MAJOR into per-(dst core, 4-block group) PSUM
    tiles via per-chunk 0/1 assignment matmuls:
        psumT[f, dst] += msg_chunk[e, f]^T  @  M_chunk[e, dst_window]
    (lhsT = the gathered chunk, rhs = a host-built 0/1 matrix).  PE sums
    are deterministic -- dma_scatter_add would race on duplicate dst rows
    on real hardware (verified experimentally).
  * Self-loop term rides along as synthetic (j -> j) edges.
  * Partial slabs go to a [8*128, npc] bf16 accumulator; ONE bf16
    ReduceScatter (priced by OUT size = 1.6MB -> ~57us vs 253us AllGather)
    hands each core its final aggregated feature-major slab.
  * Epilogue: scale by s (column-wise, feature-major), then lhsT IS already
    transposed for the Wcat matmul -- no per-block PE transposes needed.

Static SPMD choreography: chunk counts, matmul windows, and M offsets are
max-over-cores static geometry; all per-core variation lives in tensor
contents (gather idx, M values), with pad positions pointing at row 0 and
zero M columns.

Gather calls carry <=896 idxs (hw SWDGE descriptor ring holds 128 in-flight
entries per engine; larger calls hang the device).  Calls rotate over 4
SWDGE queues; queue_num is re-derived post-compile from the Tile-assigned
DMASW sem lane.
"""

import numpy as np

N_NODES = 50000
N_EDGES = 800000
D_IN = 128
D_HID = 128
D_LAT = 64
BN_EPS = 1e-5
N_CORES = 8
P = 128
LO_CORES = 5  # cores 0..4 form the "lo" table half; 5*6272=31360 < 32768
              # (dma_gather int16 indices address at most 32768 rows per call)

CALL_COLS = 7      # gather call size: 7 cols * 128 = 896 idxs (hw ring cap)
NUM_Q = 4          # SWDGE queues
GB = 4             # dst blocks per PSUM group (layer 2 push)

_CACHE = {}


# ----------------------------------------------------------------------------
# Host-side preprocessing
# ----------------------------------------------------------------------------


def _wrap_idx(lin):
    """dma_gather idx layout: position i -> [i%16, i//16], replicated to 128
    partitions. lin: [n] int array (n % 16 == 0) -> [128, n//16] int16."""
    n = lin.shape[0]
    w = lin.reshape(n // 16, 16).T.astype(np.int16)  # [16, n//16]
    return np.tile(w, (8, 1))


def _pack_calls(D, call_cols):
    """Slice the global column space into calls of <= call_cols columns.

    A call may cover partial blocks; each call carries its piece list
    [(block, col_off_in_call, width, first, last)].
    """
    C0 = np.concatenate([[0], np.cumsum(D)]).astype(np.int64)
    ct = int(C0[-1])
    calls = []
    for c0 in range(0, ct, call_cols):
        c1 = min(c0 + call_cols, ct)
        pieces = []
        for b in range(len(D)):
            lo = max(c0, int(C0[b]))
            hi = min(c1, int(C0[b + 1]))
            if lo < hi:
                pieces.append(
                    (b, lo - c0, hi - lo, lo == int(C0[b]), hi == int(C0[b + 1]))
                )
        calls.append((c0, c1 - c0, tuple(pieces)))
    return C0, calls


def _build_pass(tcoord_src, tkey_dst, n_cores, npc, blocks, call_cols,
                pad_idx, idx_base):
    """Build one gather pass layout (layer-1 pull).

    tcoord_src: per-edge source table coord (already offset for hi pass)
    tkey_dst:   per-edge dst node key in THIS pass's permutation
    Returns D [blocks], C0, calls, idx arrays [n_cores, 128, c_total] int32.
    """
    deg = np.bincount(tkey_dst, minlength=n_cores * npc)
    d3 = deg.reshape(n_cores, blocks, P)
    D = d3.max(axis=(0, 2)).astype(np.int64)
    D = np.maximum(D, 1)
    C0, calls = _pack_calls(D, call_cols)
    c_total = int(C0[-1])

    idx = np.full((n_cores, P, c_total), pad_idx - idx_base, dtype=np.int32)
    eorder = np.argsort(tkey_dst, kind="stable")
    k_s = tkey_dst[eorder]
    src_s = (tcoord_src[eorder] - idx_base).astype(np.int32)
    grp = np.searchsorted(k_s, k_s)
    slot = np.arange(k_s.size) - grp
    core_e = k_s // npc
    local_e = k_s % npc
    b_e = local_e // P
    p_e = local_e % P
    col_e = C0[b_e] + slot
    assert (slot < D[b_e]).all()
    idx[core_e, p_e, col_e] = src_s
    return D, C0, calls, idx, c_total


def _idx_to_wrapped(idx):
    """[n_cores, 128, c_total] int32 -> wrapped int16 [n_cores, 128, 8*c_total].

    Global linear position order is column-major (i = c*128 + p); contiguous
    position chunks map to contiguous wrapped columns, so any call covering
    cols [c0, c1) reads the wrapped slice [:, 8*c0 : 8*c1]."""
    n_cores, _, c_total = idx.shape
    out = np.empty((n_cores, 128, 8 * c_total), dtype=np.int16)
    for k in range(n_cores):
        lin = idx[k].T.reshape(-1)
        out[k] = _wrap_idx(lin)
    return out


def _plan_l2(src, dst, core_of, local_of, l2col_of, node_of, npc, blocks,
             s_l2):
    """Layer-2 push plan: per-src-core edge streams, static chunk/window
    choreography, per-core gather idx + 0/1 M matrices.

    The dst COLUMN order is a separate per-core permutation (l2col_of,
    sorted by total degree) -- balanced across src cores, unlike the main
    dlo-sorted layout, so the max-over-cores chunk envelope stays tight.

    Static geometry (identical across cores, SPMD):
      groups: (kk, g, g0, W, C, m_off, m_cols, windows=((w0, w1, mo), ...))
    Per-core data: idx stream (int, gather rows into tab2loc), M [128, Mtot].
    """
    n_cores = N_CORES
    GR = (blocks + GB - 1) // GB
    group_w = [min(GB * P, npc - g * GB * P) for g in range(GR)]

    # per-core edge lists sorted by (dst_core, dst_l2pos).  Self loops are
    # NOT included here -- they would land only on the own-core stream and
    # blow up the max-over-cores static envelope by ~n_local per dst core;
    # the self term is added in the epilogue from a transposed local gather
    # that runs during the ReduceScatter wait.
    ecore = []
    for k in range(n_cores):
        m = core_of[src] == k
        s_loc = local_of[src[m]]
        d_core = core_of[dst[m]]
        d_loc = l2col_of[dst[m]]
        key = d_core * npc + d_loc
        o = np.argsort(key, kind="stable")
        ecore.append((s_loc[o], d_core[o], d_loc[o], key[o]))

    # group slices per core: searchsorted bounds on key
    # chunk counts per (kk, g): ceil(max_core n / 128)
    groups = []
    per_core_chunks = [[] for _ in range(n_cores)]  # list of (idx128, jrel128)
    m_off = 0
    order_kg = [(kk, g) for kk in range(n_cores) for g in range(GR)]
    for kk, g in order_kg:
        if True:
            g0 = g * GB * P
            W = group_w[g]
            lo_key = kk * npc + g0
            hi_key = kk * npc + g0 + W
            segs = []
            for k in range(n_cores):
                keys = ecore[k][3]
                a = np.searchsorted(keys, lo_key)
                b = np.searchsorted(keys, hi_key)
                segs.append((a, b))
            nmax = max(b - a for a, b in segs)
            C = (nmax + P - 1) // P
            windows = []
            g_m0 = m_off
            for c in range(C):
                w0, w1 = W, 0
                for k in range(n_cores):
                    a, b = segs[k]
                    r0, r1 = a + c * P, min(a + (c + 1) * P, b)
                    if r0 < r1:
                        j = ecore[k][2][r0:r1] - g0
                        w0 = min(w0, int(j.min()))
                        w1 = max(w1, int(j.max()) + 1)
                if w1 <= w0:
                    w0, w1 = 0, 1
                windows.append((w0, w1, m_off - g_m0))
                m_off += w1 - w0
            groups.append((kk, g, g0, W, C, g_m0, m_off - g_m0,
                           tuple(windows)))
            for k in range(n_cores):
                a, b = segs[k]
                for c in range(C):
                    r0, r1 = a + c * P, min(a + (c + 1) * P, b)
                    n = max(0, r1 - r0)
                    idx128 = np.zeros(P, np.int32)
                    jrel = np.full(P, -1, np.int32)
                    if n > 0:
                        sl = ecore[k][0][r0:r1]
                        idx128[:n] = (sl % P) * blocks + sl // P
                        jrel[:n] = ecore[k][2][r0:r1] - g0
                    per_core_chunks[k].append((idx128, jrel))

    total_chunks = sum(gr[4] for gr in groups)
    m_total = m_off
    n_calls = (total_chunks + CALL_COLS - 1) // CALL_COLS

    # per-core tensors
    widx2 = []
    Ms = []
    for k in range(n_cores):
        lin = np.concatenate([c[0] for c in per_core_chunks[k]])
        widx2.append(_wrap_idx(lin))
        M = np.zeros((P, m_total), np.float32)
        t = 0
        for kk, g, g0, W, C, g_m0, g_mc, windows in groups:
            for c in range(C):
                idx128, jrel = per_core_chunks[k][t]
                w0, w1, mo = windows[c]
                rows = np.nonzero(jrel >= 0)[0]
                # M carries the dst-side s factor (out = s * sum), so the
                # epilogue needs no post-matmul scaling at all.
                np.add.at(M, (rows, g_m0 + mo + (jrel[rows] - w0)),
                          s_l2[kk][g0 + jrel[rows]])
                t += 1
        Ms.append(M)

    max_mw = max((gr[6] for gr in groups), default=1)
    max_win = max((w1 - w0 for gr in groups for (w0, w1, _) in gr[7]),
                  default=1)
    return dict(
        groups=tuple(groups),
        total_chunks=total_chunks,
        m_total=m_total,
        n_calls=n_calls,
        max_mw=max_mw,
        max_win=max_win,
        widx2=widx2,
        Ms=Ms,
    )


def _plan(edge_index, n_nodes, n_cores, call_cols):
    src = np.asarray(edge_index[0], dtype=np.int64)
    dst = np.asarray(edge_index[1], dtype=np.int64)

    deg_in = np.bincount(dst, minlength=n_nodes).astype(np.int64)
    s = (1.0 / np.sqrt((deg_in + 1).astype(np.float64))).astype(np.float32)

    n_local = (n_nodes + n_cores - 1) // n_cores
    blocks = (n_local + 1 + P - 1) // P
    npc = blocks * P
    lo_rows = LO_CORES * npc
    assert lo_rows < 32768 and (n_cores * npc - lo_rows) < 32768

    # ---- core assignment: deal by total-degree rank (balances edge load and
    # aligns block-degree profiles across cores).
    order = np.argsort(-deg_in, kind="stable")
    rank_of = np.empty(n_nodes, dtype=np.int64)
    rank_of[order] = np.arange(n_nodes)
    core_of = rank_of % n_cores

    src_is_lo = core_of[src] < LO_CORES
    dlo = np.bincount(dst[src_is_lo], minlength=n_nodes)
    dhi = np.bincount(dst[~src_is_lo], minlength=n_nodes)

    # main layout: per-core locals sorted by lo-degree (tight LO padding)
    local_of = np.empty(n_nodes, dtype=np.int64)
    node2hi = np.empty(n_nodes, dtype=np.int64)
    for k in range(n_cores):
        nodes_k = np.nonzero(core_of == k)[0]
        o = nodes_k[np.argsort(-dlo[nodes_k], kind="stable")]
        local_of[o] = np.arange(o.size)
        o2 = nodes_k[np.argsort(-dhi[nodes_k], kind="stable")]
        node2hi[o2] = k * npc + np.arange(o2.size)
    node2table = core_of * npc + local_of

    # ---- gather-source row numbering: tables are [(core,part), (block,feat)]
    # 2-D tensors, so node (core k, local j=b*128+p) lives at flat row
    # (k*128+p)*blocks + b of its half (hi half: k-LO_CORES).
    def kpb_row(core, local, core0):
        return ((core - core0) * P + local % P) * blocks + local // P

    node2row = np.where(
        core_of < LO_CORES,
        kpb_row(core_of, local_of, 0),
        kpb_row(core_of, local_of, LO_CORES),
    )
    pad_lo = kpb_row(0, npc - 1, 0)
    pad_hi = kpb_row(N_CORES - 1, npc - 1, LO_CORES)

    # ---- LO pass on the main permutation
    D_lo, C0_lo, calls_lo, idx_lo, ct_lo = _build_pass(
        node2row[src[src_is_lo]], node2table[dst[src_is_lo]], n_cores, npc,
        blocks, call_cols, pad_lo, 0,
    )

    # ---- HI pass on the hi permutation
    D_hi, C0_hi, calls_hi, idx_hi, ct_hi = _build_pass(
        node2row[src[~src_is_lo]], node2hi[dst[~src_is_lo]], n_cores, npc,
        blocks, call_cols, pad_hi, 0,
    )

    # ---- combine map: main-layout local j gets acc_hi[himap[j]] added
    himap = np.full((n_cores, npc), npc - 1, dtype=np.int64)  # pad -> pad row
    for k in range(n_cores):
        nodes_k = np.nonzero(core_of == k)[0]
        himap[k, local_of[nodes_k]] = node2hi[nodes_k] % npc

    widx_lo = _idx_to_wrapped(idx_lo)
    widx_hi = _idx_to_wrapped(idx_hi)
    himap_row = (himap % P) * blocks + himap // P
    widx_cb = np.stack([_wrap_idx(himap_row[k]) for k in range(n_cores)])

    # per-core node lists and s in the MAIN layout
    node_of = np.full((n_cores, npc), -1, dtype=np.int64)
    s_arr = np.zeros((n_cores, P, blocks), dtype=np.float32)
    for k in range(n_cores):
        nodes_k = np.nonzero(core_of == k)[0]
        loc = local_of[nodes_k]
        node_of[k, loc] = nodes_k
        s_arr[k, loc % P, loc // P] = s[nodes_k]

    # layer-2 dst column permutation: per-core total-degree sort (balanced
    # per-src-core edge counts -> tight static chunk envelope)
    l2col_of = np.empty(n_nodes, dtype=np.int64)
    node_of_l2 = np.full((n_cores, npc), -1, dtype=np.int64)
    s_l2 = np.zeros((n_cores, npc), dtype=np.float32)
    wself = []
    for k in range(n_cores):
        nodes_k = np.nonzero(core_of == k)[0]
        o = nodes_k[np.argsort(-deg_in[nodes_k], kind="stable")]
        l2col_of[o] = np.arange(o.size)
        node_of_l2[k, : o.size] = o
        s_l2[k, : o.size] = s[o]
        rows = np.zeros(npc, dtype=np.int64)
        loc = local_of[o]
        rows[: o.size] = (loc % P) * blocks + loc // P
        wself.append(_wrap_idx(rows))

    l2 = _plan_l2(src, dst, core_of, local_of, l2col_of, node_of, npc, blocks,
                  s_l2)

    return dict(
        s=s,
        node2table=node2table,
        node_of=node_of,
        npc=npc,
        blocks=blocks,
        n_local=n_local,
        lo_rows=lo_rows,
        s_arr=s_arr,
        D_lo=D_lo, C0_lo=C0_lo, calls_lo=calls_lo, idx_lo=idx_lo, ct_lo=ct_lo,
        D_hi=D_hi, C0_hi=C0_hi, calls_hi=calls_hi, idx_hi=idx_hi, ct_hi=ct_hi,
        himap=himap,
        widx_lo=widx_lo, widx_hi=widx_hi, widx_cb=widx_cb,
        node_of_l2=node_of_l2, s_l2=s_l2, wself=wself,
        l2=l2,
    )


def _to_bf16(a):
    import ml_dtypes

    return a.astype(ml_dtypes.bfloat16)


def _host_inputs(plan, x, W1, Wmu, Wls, bmu, bls, gamma, beta):
    npc = plan["npc"]
    node_of = plan["node_of"]
    s = plan["s"]
    l2 = plan["l2"]
    wcat = np.concatenate([Wmu, Wls], axis=1).astype(np.float32)
    bcat = np.concatenate([bmu, bls]).astype(np.float32).reshape(D_HID, 1)

    # s-prescaled x in the partition-major table layout:
    # row (core*128 + p), col (block*128 + f) holds node (core, b*128+p).
    blocks = npc // P
    xs_tab = np.zeros((N_CORES * npc, x.shape[1]), dtype=np.float32)
    for k in range(N_CORES):
        nodes = node_of[k]
        valid = nodes >= 0
        xs_tab[k * npc + np.nonzero(valid)[0]] = (
            x[nodes[valid]] * s[nodes[valid]][:, None]
        )
    xs_kpb = (
        xs_tab.reshape(N_CORES, blocks, P, D_IN)
        .transpose(0, 2, 1, 3)
        .reshape(N_CORES * P, blocks * D_IN)
    )
    xs_kpb = _to_bf16(xs_kpb)
    xs_lo = np.ascontiguousarray(xs_kpb[: LO_CORES * P])
    xs_hi = np.ascontiguousarray(xs_kpb[LO_CORES * P :])

    per_core = []
    for k in range(N_CORES):
        sa = plan["s_arr"][k]  # [P, blocks]
        # layer-2 epilogue scale: s per output partition (L2 column order:
        # block b, partition p <-> l2 position b*128+p)
        scol2 = np.ascontiguousarray(
            plan["s_l2"][k].reshape(blocks, P).T.astype(np.float32))
        per_core.append(
            {
                "xslo": xs_lo,
                "xshi": xs_hi,
                "xso": np.ascontiguousarray(xs_kpb[k * P : (k + 1) * P]),
                "W1": np.ascontiguousarray(_to_bf16(W1)),
                "Wcat": np.ascontiguousarray(_to_bf16(wcat)),
                "bcat": bcat,
                "s_arr": np.ascontiguousarray(sa),
                "widx_lo": np.ascontiguousarray(plan["widx_lo"][k]),
                "widx_hi": np.ascontiguousarray(plan["widx_hi"][k]),
                "widx_cb": np.ascontiguousarray(plan["widx_cb"][k]),
                "gamma": gamma.astype(np.float32).reshape(D_HID, 1),
                "beta": beta.astype(np.float32).reshape(D_HID, 1),
                "widx2": np.ascontiguousarray(l2["widx2"][k]),
                "M2": np.ascontiguousarray(_to_bf16(l2["Ms"][k])),
                "scol2": scol2,
                "wself": np.ascontiguousarray(plan["wself"][k]),
            }
        )
    return per_core


def _postprocess(plan, outs):
    n_nodes = int(plan["node_of"].max()) + 1
    mu = np.zeros((n_nodes, D_LAT), dtype=np.float32)
    ls = np.zeros((n_nodes, D_LAT), dtype=np.float32)
    node_of = plan["node_of_l2"]  # out_cat columns are in the L2 permutation
    npc = node_of.shape[1]
    blocks = npc // 128
    for k in range(N_CORES):
        nodes = node_of[k]
        valid = nodes >= 0
        o = np.asarray(outs[k]).astype(np.float32).reshape(128, blocks, 128)
        o = o.transpose(1, 0, 2).reshape(npc, 128)  # node-major (b*128+p)
        mu[nodes[valid]] = o[valid.nonzero()[0], :D_LAT]
        ls[nodes[valid]] = o[valid.nonzero()[0], D_LAT:]
    return mu, ls


# ----------------------------------------------------------------------------
# Device program
# ----------------------------------------------------------------------------


def _build_program(geom):
    from concourse import bacc, bass, mybir, tile
    from concourse.masks import make_identity

    (npc, blocks, D_lo, calls_lo, ct_lo, D_hi, calls_hi, ct_hi, n_real,
     call_cols, lo_rows, l2g) = geom
    (groups, total_chunks, m_total, n_calls, max_mw, max_win) = l2g
    D_lo, D_hi = list(D_lo), list(D_hi)
    f32 = mybir.dt.float32
    bf16 = mybir.dt.bfloat16
    i16 = mybir.dt.int16
    GR = (blocks + GB - 1) // GB

    nc = bacc.Bacc("TRN2", target_bir_lowering=False, debug=False,
                   num_devices=N_CORES, num_swdge_queues=NUM_Q)

    t_xslo = nc.dram_tensor("xslo", [LO_CORES * P, npc], bf16,
                            kind="ExternalInput")
    t_xshi = nc.dram_tensor("xshi", [(N_CORES - LO_CORES) * P, npc], bf16,
                            kind="ExternalInput")
    t_xso = nc.dram_tensor("xso", [P, npc], bf16, kind="ExternalInput")
    t_W1 = nc.dram_tensor("W1", [P, D_HID], bf16, kind="ExternalInput")
    t_Wcat = nc.dram_tensor("Wcat", [D_HID, P], bf16, kind="ExternalInput")
    t_bcat = nc.dram_tensor("bcat", [P, 1], f32, kind="ExternalInput")
    t_sarr = nc.dram_tensor("s_arr", [P, blocks], f32, kind="ExternalInput")
    t_wlo = nc.dram_tensor("widx_lo", [P, 8 * ct_lo], i16, kind="ExternalInput")
    t_whi = nc.dram_tensor("widx_hi", [P, 8 * ct_hi], i16, kind="ExternalInput")
    t_wcb = nc.dram_tensor("widx_cb", [P, npc // 16], i16, kind="ExternalInput")
    t_gamma = nc.dram_tensor("gamma", [D_HID, 1], f32, kind="ExternalInput")
    t_beta = nc.dram_tensor("beta", [D_HID, 1], f32, kind="ExternalInput")
    t_widx2 = nc.dram_tensor("widx2", [P, total_chunks * P // 16], i16,
                             kind="ExternalInput")
    t_M2 = nc.dram_tensor("M2", [P, m_total], bf16, kind="ExternalInput")
    t_scol2 = nc.dram_tensor("scol2", [P, blocks], f32, kind="ExternalInput")
    t_wself = nc.dram_tensor("wself", [P, npc // 16], i16,
                             kind="ExternalInput")
    t_out = nc.dram_tensor("out_cat", [P, npc], bf16, kind="ExternalOutput")

    tab2loc = nc.dram_tensor("tab2loc", [npc, P], bf16)
    acc1 = nc.dram_tensor("acc_hi1", [P, npc], bf16)
    accL2 = nc.dram_tensor("accL2", [N_CORES * P, npc], bf16)
    rs_out = nc.dram_tensor("rs_out", [P, npc], bf16)
    st_in = nc.dram_tensor("st_in", [2 * D_HID], f32)
    st_out = nc.dram_tensor("st_out", [N_CORES * 2 * D_HID], f32,
                            addr_space="Shared")

    groups_rep = [list(range(N_CORES))]
    inv_n = 1.0 / float(n_real)

    with tile.TileContext(nc) as tc:
        with (
            tc.tile_pool(name="persist", bufs=1) as persist,
            tc.tile_pool(name="stream", bufs=4) as stream,
            tc.tile_pool(name="gath", bufs=3) as gath,
            tc.tile_pool(name="gath2", bufs=4) as gath2,
            tc.tile_pool(name="mpool", bufs=3) as mpool,
            tc.tile_pool(name="ppool", bufs=3) as ppool,
            tc.tile_pool(name="gathcb", bufs=1) as gathcb,
            tc.tile_pool(name="small", bufs=1) as small,
            tc.tile_pool(name="ps", bufs=2, space="PSUM") as psp,
            tc.tile_pool(name="ps_blk", bufs=3, space="PSUM") as psblk,
            tc.tile_pool(name="ps_acc", bufs=1, space="PSUM") as psacc,
        ):
            H = persist.tile([P, npc], bf16, tag="H")
            ownslab = persist.tile([P, npc], bf16, tag="ownslab")
            accsb = persist.tile([P, npc], bf16, tag="accsb")
            outsb = persist.tile([P, npc], bf16, tag="outsb")
            wlo = persist.tile([P, 8 * ct_lo], i16, tag="wlo")
            whi = persist.tile([P, 8 * ct_hi], i16, tag="whi")
            wcb = persist.tile([P, npc // 16], i16, tag="wcb")
            widx2 = persist.tile([P, total_chunks * P // 16], i16,
                                 tag="widx2")
            wself = persist.tile([P, npc // 16], i16, tag="wself")
            selfT = persist.tile([P, npc], bf16, tag="selfT")
            w1 = small.tile([P, D_HID], bf16, tag="w1")
            wcat = small.tile([D_HID, P], bf16, tag="wcat")
            sarr = small.tile([P, blocks], f32, tag="sarr")
            scol2 = small.tile([P, blocks], f32, tag="scol2")
            gcol = small.tile([D_HID, 1], f32, tag="gcol")
            bcol = small.tile([D_HID, 1], f32, tag="bcol")
            bccol = small.tile([P, 1], f32, tag="bccol")
            ident = small.tile([P, P], bf16, tag="ident")
            identf = small.tile([P, P], f32, tag="identf")
            zeroM = small.tile([P, GB * P], bf16, tag="zeroM")
            ones_col = small.tile([P, 1], bf16, tag="ones_col")
            ones_row = small.tile([1, P], f32, tag="ones_row")
            grep = small.tile([P, P], f32, tag="grep")
            brep = small.tile([P, P], f32, tag="brep")
            grep4 = small.tile([P, 4 * P], bf16, tag="grep4")
            brep4 = small.tile([P, 4 * P], bf16, tag="brep4")
            bcrep = small.tile([P, P], f32, tag="bcrep")
            bcrep4 = small.tile([P, 4 * P], f32, tag="bcrep4")

            nc.sync.dma_start(out=whi[:], in_=t_whi[:])
            nc.sync.dma_start(out=wlo[:], in_=t_wlo[:])
            nc.sync.dma_start(out=wcb[:], in_=t_wcb[:])
            nc.sync.dma_start(out=widx2[:], in_=t_widx2[:])
            nc.sync.dma_start(out=wself[:], in_=t_wself[:])
            nc.sync.dma_start(out=scol2[:], in_=t_scol2[:])
            nc.sync.dma_start(out=w1[:], in_=t_W1[:])
            nc.sync.dma_start(out=wcat[:], in_=t_Wcat[:])
            nc.sync.dma_start(out=sarr[:], in_=t_sarr[:])
            nc.sync.dma_start(out=gcol[:], in_=t_gamma[:])
            nc.sync.dma_start(out=bcol[:], in_=t_beta[:])
            nc.sync.dma_start(out=bccol[:], in_=t_bcat[:])
            make_identity(nc, ident[:])
            make_identity(nc, identf[:])
            nc.vector.memset(zeroM[:], 0.0)
            nc.vector.memset(ones_col[:], 1.0)
            nc.vector.memset(ones_row[:], 1.0)
            warm = small.tile([1, 1], f32, tag="warm")
            nc.vector.memset(warm[:], 1.0)
            nc.scalar.activation(out=warm[:], in_=warm[:],
                                 func=mybir.ActivationFunctionType.Sqrt)

            def outer_bcast(col_ap, dst_tile):
                pst = psp.tile([P, P], f32, space="PSUM", tag="ps_big")
                nc.tensor.transpose(out=pst[0:1, :], in_=col_ap,
                                    identity=identf[:])
                row = stream.tile([1, P], f32, tag="rowbuf")
                nc.vector.tensor_copy(out=row[:], in_=pst[0:1, :])
                psb = psp.tile([P, P], f32, space="PSUM", tag="ps_big")
                nc.tensor.matmul(out=psb[:], lhsT=ones_row[:], rhs=row[:],
                                 start=True, stop=True)
                nc.vector.tensor_copy(out=dst_tile[:], in_=psb[:])

            outer_bcast(bccol[:], bcrep)
            for r in range(4):
                nc.vector.tensor_copy(out=bcrep4[:, r * P : (r + 1) * P],
                                      in_=bcrep[:])

            # --- layer 1 gathers raw (s*x): A(x@W1) == A(x)@W1, so W1 is
            # applied per-block AFTER aggregation; the host-marshaled xslo/
            # xshi inputs ARE the gather tables (no stage-1, no table write).
            nc.sync.dma_start(out=ownslab[:], in_=t_xso[:, :])

            ps_stats = psacc.tile([D_HID, 2], f32, space="PSUM",
                                  tag="ps_stats")

            qn = [0]

            def next_q():
                q = qn[0]
                qn[0] = (qn[0] + 1) % NUM_Q
                return q

            def gather(out_ap, table_ap, widx_ap, n_idx):
                nc.gpsimd.dma_gather(
                    out_ap.rearrange("p (c f) -> p c f", f=P),
                    table_ap,
                    widx_ap,
                    num_idxs=n_idx,
                    num_idxs_reg=n_idx,
                    elem_size=P,
                    queue_num=next_q(),
                )

            def aggregate(tlo, thi, acc_t, selfslab):
                """Layer-1 pull aggregation -> H blocks + BN stats."""
                grp = {}

                def blk_view(b, pre):
                    g, r = b // GB, b % GB
                    if g not in grp:
                        grp[g] = psblk.tile([P, GB * P], f32, space="PSUM",
                                            tag="ps_blk",
                                            name=f"{pre}_{g}")
                    return grp[g][:, r * P : (r + 1) * P]

                g_open = set()

                def g_last(b):
                    return min((b // GB) * GB + GB - 1, blocks - 1)

                def flags(b, is_first_mm, is_last_mm):
                    g = b // GB
                    start = is_first_mm and g not in g_open
                    if start:
                        g_open.add(g)
                    stop = is_last_mm and b == g_last(b)
                    return start, stop

                for c0, cols, pieces in calls_hi:
                    gt = gath.tile([P, call_cols * P], bf16, tag="gt")
                    gather(gt[:, : cols * P],
                           thi.rearrange("q (r f) -> (q r) f", f=P),
                           whi[:, 8 * c0 : 8 * (c0 + cols)], cols * P)
                    for b, o, d, first, last in pieces:
                        bv = blk_view(b, "psg_hi")
                        for i in range(d):
                            st_, sp_ = flags(b, first and i == 0,
                                             last and i == d - 1)
                            nc.tensor.matmul(
                                out=bv, lhsT=ident[:],
                                rhs=gt[:, (o + i) * P : (o + i + 1) * P],
                                start=st_, stop=sp_)
                        if last and b == g_last(b):
                            g = b // GB
                            w = (b % GB) + 1
                            nc.vector.tensor_copy(
                                out=accsb[:, g * GB * P : g * GB * P + w * P],
                                in_=grp.pop(g)[:, : w * P])
                nc.sync.dma_start(out=acc_t[:, :], in_=accsb[:])

                # combine gathers (cbt[:, j] = acc_t[himap[j]]) are issued
                # a few calls INTO the lo stream so the acc-write latency
                # hides behind lo gather work.
                cbt = gathcb.tile([P, npc], bf16, tag="gtcb")

                grp.clear()
                g_open.clear()

                def start_block(b):
                    bv = blk_view(b, "psg_lo")
                    st_, _ = flags(b, True, False)
                    nc.tensor.matmul(out=bv, lhsT=ident[:],
                                     rhs=selfslab[:, b * P : (b + 1) * P],
                                     start=st_, stop=False)
                    return bv

                cb_at = min(1, (D_lo[0] - 1) // call_cols)
                for ci, (c0, cols, pieces) in enumerate(calls_lo):
                    if ci == cb_at:
                        for cb0 in range(0, blocks, call_cols):
                            cb1 = min(cb0 + call_cols, blocks)
                            gather(cbt[:, cb0 * P : cb1 * P],
                                   acc_t.rearrange("q (r f) -> (q r) f", f=P),
                                   wcb[:, 8 * cb0 : 8 * cb1],
                                   (cb1 - cb0) * P)
                    gt = gath.tile([P, call_cols * P], bf16, tag="gt")
                    gather(gt[:, : cols * P],
                           tlo.rearrange("q (r f) -> (q r) f", f=P),
                           wlo[:, 8 * c0 : 8 * (c0 + cols)], cols * P)
                    for b, o, d, first, last in pieces:
                        if first:
                            start_block(b)
                        bv = blk_view(b, "psg_lo")
                        for i in range(d):
                            nc.tensor.matmul(
                                out=bv, lhsT=ident[:],
                                rhs=gt[:, (o + i) * P : (o + i + 1) * P],
                                start=False, stop=False)
                        if last:
                            _, sp_ = flags(b, False, True)
                            nc.tensor.matmul(
                                out=bv, lhsT=ident[:],
                                rhs=cbt[:, b * P : (b + 1) * P],
                                start=False, stop=sp_)
                        if last and b == g_last(b):
                            g = b // GB
                            gt_ps = grp.pop(g)
                            for r in range(b % GB + 1):
                                bb = g * GB + r
                                sl = slice(bb * P, (bb + 1) * P)
                                # U = s*(agg); h1 = U @ W1
                                ub = stream.tile([P, P], bf16, tag="ub")
                                nc.vector.tensor_scalar_mul(
                                    ub[:], gt_ps[:, r * P : (r + 1) * P],
                                    sarr[:, bb : bb + 1])
                                psT = psp.tile([P, P], bf16, space="PSUM",
                                               tag="ps_bigT")
                                nc.tensor.transpose(out=psT[:], in_=ub[:],
                                                    identity=ident[:])
                                uT = stream.tile([P, P], bf16, tag="uT")
                                nc.vector.tensor_copy(out=uT[:], in_=psT[:])
                                ps1 = psp.tile([P, P], f32, space="PSUM",
                                               tag="ps_big")
                                nc.tensor.matmul(out=ps1[:], lhsT=uT[:],
                                                 rhs=w1[:], start=True,
                                                 stop=True)
                                nc.vector.tensor_copy(out=H[:, sl],
                                                      in_=ps1[:])
                                sq = stream.tile([P, D_HID], bf16,
                                                 tag="sq")
                                nc.scalar.square(out=sq[:], in_=H[:, sl])
                                nc.tensor.matmul(
                                    out=ps_stats[:, 0:1], lhsT=H[:, sl],
                                    rhs=ones_col[:],
                                    start=(bb == 0), stop=False)
                                nc.tensor.matmul(
                                    out=ps_stats[:, 1:2], lhsT=sq[:],
                                    rhs=ones_col[:],
                                    start=False,
                                    stop=(bb == blocks - 1))

            aggregate(t_xslo, t_xshi, acc1, ownslab)

            # --- BN ---------------------------------------------------------
            st = small.tile([D_HID, 2], f32, tag="st")
            nc.vector.tensor_copy(out=st[:], in_=ps_stats[:])
            nc.sync.dma_start(out=st_in[:], in_=st[:])
            nc.gpsimd.collective_compute(
                "AllGather", mybir.AluOpType.bypass,
                replica_groups=groups_rep,
                ins=[st_in[:]], outs=[st_out[:]],
            )
            st8 = small.tile([D_HID, 2 * N_CORES], f32, tag="st8")
            nc.sync.dma_start(
                out=st8.rearrange("p (c t) -> p c t", t=2),
                in_=st_out[:].rearrange("(c p t) -> p c t", p=D_HID, t=2),
            )
            st2 = small.tile([D_HID, 2], f32, tag="st2")
            nc.vector.reduce_sum(
                out=st2[:],
                in_=st8.rearrange("p (c t) -> p t c", t=2),
                axis=mybir.AxisListType.X)

            eps_col = small.tile([D_HID, 1], f32, tag="eps_col")
            nc.vector.memset(eps_col[:], BN_EPS)
            mean = small.tile([D_HID, 1], f32, tag="mean")
            msq = small.tile([D_HID, 1], f32, tag="msq")
            var = small.tile([D_HID, 1], f32, tag="var")
            std = small.tile([D_HID, 1], f32, tag="std")
            istd = small.tile([D_HID, 1], f32, tag="istd")
            gp = small.tile([D_HID, 1], f32, tag="gp")
            bp_ = small.tile([D_HID, 1], f32, tag="bp")
            nc.vector.tensor_scalar_mul(mean[:], st2[:, 0:1], inv_n)
            nc.vector.tensor_scalar_mul(msq[:], st2[:, 1:2], inv_n)
            nc.scalar.square(out=var[:], in_=mean[:])
            nc.vector.tensor_tensor(out=var[:], in0=msq[:], in1=var[:],
                                    op=mybir.AluOpType.subtract)
            nc.scalar.activation(out=std[:], in_=var[:],
                                 func=mybir.ActivationFunctionType.Sqrt,
                                 bias=eps_col[:])
            nc.vector.reciprocal(out=istd[:], in_=std[:])
            nc.vector.tensor_tensor(out=gp[:], in0=gcol[:], in1=istd[:],
                                    op=mybir.AluOpType.mult)
            nc.vector.tensor_tensor(out=bp_[:], in0=mean[:], in1=gp[:],
                                    op=mybir.AluOpType.mult)
            nc.vector.tensor_tensor(out=bp_[:], in0=bcol[:], in1=bp_[:],
                                    op=mybir.AluOpType.subtract)
            outer_bcast(gp[:], grep)
            outer_bcast(bp_[:], brep)
            for r in range(4):
                nc.vector.tensor_copy(out=grep4[:, r * P : (r + 1) * P],
                                      in_=grep[:])
                nc.vector.tensor_copy(out=brep4[:, r * P : (r + 1) * P],
                                      in_=brep[:])

            # BN affine + ReLU + s-scale, written to local DRAM table
            # (tab2loc rows = p*blocks + b -- the layer-2 gather source).
            t2v = tab2loc[:, :].rearrange("(p b) f -> p b f", b=blocks)
            for b0 in range(0, blocks, 4):
                b1 = min(b0 + 4, blocks)
                w = (b1 - b0) * P
                t1 = stream.tile([P, 4 * P], bf16, tag="bn1")
                nc.vector.tensor_tensor(out=t1[:, :w], in0=H[:, b0 * P:b0 * P + w],
                                        in1=grep4[:, :w],
                                        op=mybir.AluOpType.mult)
                nc.vector.tensor_tensor(out=t1[:, :w], in0=t1[:, :w],
                                        in1=brep4[:, :w],
                                        op=mybir.AluOpType.add)
                nc.scalar.activation(out=t1[:, :w], in_=t1[:, :w],
                                     func=mybir.ActivationFunctionType.Relu)
                t2 = stream.tile([P, 4 * P], bf16, tag="bn2")
                for b in range(b0, b1):
                    r = b - b0
                    nc.vector.tensor_scalar_mul(
                        t2[:, r * P : (r + 1) * P],
                        t1[:, r * P : (r + 1) * P], sarr[:, b : b + 1])
                nc.sync.dma_start(out=t2v[:, b0:b1, :],
                                  in_=t2[:, :w].rearrange(
                                      "p (b f) -> p b f", f=P))

            # --- layer 2: push aggregation (feature-major M-matmuls) -------
            chunk_t = 0
            issued = {}

            def ensure_call(t):
                ci = t // CALL_COLS
                if ci not in issued:
                    t0 = ci * CALL_COLS
                    cols = min(CALL_COLS, total_chunks - t0)
                    gt = gath2.tile([P, CALL_COLS * P], bf16, tag="gt2")
                    gather(gt[:, : cols * P], tab2loc[:, :],
                           widx2[:, 8 * t0 : 8 * (t0 + cols)], cols * P)
                    issued[ci] = gt
                return issued[ci], t % CALL_COLS

            def emit_self():
                # self term: gather own rows in L2 column order, scale by s
                # (per-partition, node-major), transpose to feature-major.
                # Runs while the ReduceScatter holds the Pool queue.
                for c0 in range(0, blocks, CALL_COLS):
                    c1 = min(c0 + CALL_COLS, blocks)
                    sg = stream.tile([P, CALL_COLS * P], bf16, tag="sg")
                    gather(sg[:, : (c1 - c0) * P], tab2loc[:, :],
                           wself[:, 8 * c0 : 8 * c1], (c1 - c0) * P)
                    for b in range(c0, c1):
                        sgs = stream.tile([P, P], bf16, tag="sgs")
                        nc.vector.tensor_scalar_mul(
                            sgs[:], sg[:, (b - c0) * P : (b - c0 + 1) * P],
                            scol2[:, b : b + 1])
                        psT2 = psp.tile([P, P], bf16, space="PSUM",
                                        tag="ps_bigT")
                        nc.tensor.transpose(out=psT2[:], in_=sgs[:],
                                            identity=ident[:])
                        nc.vector.tensor_copy(
                            out=selfT[:, b * P : (b + 1) * P], in_=psT2[:])

            for gi, (kk, g, g0, W, C, g_m0, g_mc, windows) in enumerate(groups):
                Mt = None
                if g_mc > 0:
                    Mt = mpool.tile([P, max_mw], bf16, tag="mt")
                    nc.scalar.dma_start(out=Mt[:, :g_mc],
                                        in_=t_M2[:, g_m0 : g_m0 + g_mc])
                ps2 = psblk.tile([P, GB * P], f32, space="PSUM", tag="ps_blk",
                                 name=f"ps2_{gi}")
                nc.tensor.matmul(out=ps2[:, :W], lhsT=ident[:],
                                 rhs=zeroM[:, :W], start=True, stop=(C == 0))
                for c in range(C):
                    gt, pos = ensure_call(chunk_t)
                    w0, w1, mo = windows[c]
                    nc.tensor.matmul(
                        out=ps2[:, w0:w1],
                        lhsT=gt[:, pos * P : (pos + 1) * P],
                        rhs=Mt[:, mo : mo + (w1 - w0)],
                        start=False, stop=(c == C - 1))
                    chunk_t += 1
                pt = ppool.tile([P, GB * P], bf16, tag="pt")
                nc.vector.tensor_copy(out=pt[:, :W], in_=ps2[:, :W])
                nc.sync.dma_start(out=accL2[kk * P : (kk + 1) * P,
                                            g0 : g0 + W],
                                  in_=pt[:, :W])
            emit_self()

            nc.gpsimd.collective_compute(
                "ReduceScatter", mybir.AluOpType.add,
                replica_groups=groups_rep,
                ins=[accL2[:]], outs=[rs_out[:]],
            )

            # --- epilogue: Wcat matmuls (feature-major lhsT, no transposes,
            # no post-scale -- M and selfT already carry s).  4 blocks share
            # one PSUM group; one DVE bias-add per group.
            def epilogue(rs_t, b0, b1, off):
                for c0 in range(b0, b1, GB):
                    c1 = min(c0 + GB, b1)
                    w = (c1 - c0) * P
                    nc.sync.dma_start(
                        out=H[:, c0 * P : c0 * P + w],
                        in_=rs_t[:, c0 * P - off : c0 * P - off + w])
                    pse = psblk.tile([P, GB * P], f32, space="PSUM",
                                     tag="ps_blk", name=f"pse_{c0}")
                    for b in range(c0, c1):
                        r = (b - c0) * P
                        nc.tensor.matmul(out=pse[:, r : r + P],
                                         lhsT=H[:, b * P : (b + 1) * P],
                                         rhs=wcat[:], start=True, stop=False)
                        nc.tensor.matmul(out=pse[:, r : r + P],
                                         lhsT=selfT[:, b * P : (b + 1) * P],
                                         rhs=wcat[:], start=False,
                                         stop=True)
                    nc.vector.tensor_tensor(out=outsb[:, c0 * P : c0 * P + w],
                                            in0=pse[:, :w],
                                            in1=bcrep4[:, :w],
                                            op=mybir.AluOpType.add)
                    nc.sync.dma_start(out=t_out[:, c0 * P : c0 * P + w],
                                      in_=outsb[:, c0 * P : c0 * P + w])

            epilogue(rs_out, 0, blocks, 0)

    nc.compile()
    if NUM_Q > 1:
        # Tile assigns SWDGE completion-sem lanes (DMASW0..7) round-robin in
        # SCHEDULED order, which differs from creation order.  Each sem lane
        # must be driven by a single SWDGE queue, so re-derive queue_num from
        # the assigned lane.
        import re as _re

        for _blk in nc.m.functions[0].blocks:
            for _inst in _blk.instructions:
                if type(_inst).__name__ == "InstDMAGatherAnt":
                    _m = _re.search(r"DMASW(\d+)_",
                                    str(_inst.sync_info.on_update[0]))
                    _inst.queue_num = int(_m.group(1)) % NUM_Q
    return nc


# ----------------------------------------------------------------------------
# Entry point
# ----------------------------------------------------------------------------

_IN_NAMES = ["xslo", "xshi", "xso", "W1", "Wcat", "bcat", "s_arr",
             "widx_lo", "widx_hi", "widx_cb", "gamma", "beta",
             "widx2", "M2", "scol2", "wself"]


def _geom(plan, call_cols):
    l2 = plan["l2"]
    l2g = (
        tuple(l2["groups"]),
        int(l2["total_chunks"]),
        int(l2["m_total"]),
        int(l2["n_calls"]),
        int(l2["max_mw"]),
        int(l2["max_win"]),
    )
    return (
        plan["npc"],
        plan["blocks"],
        tuple(int(d) for d in plan["D_lo"]),
        tuple(plan["calls_lo"]),
        plan["ct_lo"],
        tuple(int(d) for d in plan["D_hi"]),
        tuple(plan["calls_hi"]),
        plan["ct_hi"],
        int(plan["node_of"].max()) + 1,
        call_cols,
        plan["lo_rows"],
        l2g,
    )


def _run_hw(nc, per_core, trace=False, trace_cores=None):
    from concourse import bass_utils

    in_maps = [{nm: per_core[k][nm] for nm in _IN_NAMES} for k in range(N_CORES)]
    res = bass_utils.run_bass_kernel_spmd(
        nc, in_maps, core_ids=list(range(N_CORES)), trace=trace,
        trace_cores=trace_cores,
    )
    outs = [res.results[k]["out_cat"] for k in range(N_CORES)]
    return outs, res


def kernel(x, edge_index, W1, b1, gamma, beta, Wmu, bmu, Wls, bls):
    x = np.asarray(x, dtype=np.float32)
    edge_index = np.asarray(edge_index)
    W1 = np.asarray(W1, dtype=np.float32)
    gamma = np.asarray(gamma, dtype=np.float32)
    beta = np.asarray(beta, dtype=np.float32)
    Wmu = np.asarray(Wmu, dtype=np.float32)
    bmu = np.asarray(bmu, dtype=np.float32)
    Wls = np.asarray(Wls, dtype=np.float32)
    bls = np.asarray(bls, dtype=np.float32)

    plan = _plan(edge_index, x.shape[0], N_CORES, call_cols=CALL_COLS)
    per_core = _host_inputs(plan, x, W1, Wmu, Wls, bmu, bls, gamma, beta)

    geom = _geom(plan, CALL_COLS)
    if geom not in _CACHE:
        _CACHE[geom] = _build_program(geom)
    nc = _CACHE[geom]

    outs, _ = _run_hw(nc, per_core, trace=False)
    mu, ls = _postprocess(plan, outs)
    return mu, ls


# "M2" key is provided by _host_inputs; keep name mapping for clarity.


# revision 48
# speedup vs baseline: 1.0151x; 1.0065x over previous
"""GCN encoder (2x GCNConv + BatchNorm/ReLU) on 8 Trainium2 NeuronCores.

Math: with s = 1/sqrt(deg+1) (deg = in-degree by dst), the GCN edge norm
factorizes: norm_e = s[src]*s[dst], so for any node features H,
    A(H) := segsum(norm_e * H[src], dst) + H * s^2
          = s * ( segsum( (s*H)[src], dst) + (s*H) )
and GCNConv(H, W, b) = A(H)@W + b = A(H@W) + b, so the whole net needs only
TWO sparse aggregations (layer1 on (s*x)@W1, layer2 on s*post-BN hidden),
and mu / log_std share the second one.

Layer 1 (pull): by linearity A(x@W1) == A(x)@W1, the host-marshaled bf16 x
tables (replicated to every core's DRAM as inputs) ARE the gather tables --
no collective at all.  Slot-aligned gathers + identity-matmul PSUM
accumulation per dst block, then W1 applied post-aggregation.

Layer 2 (push + ReduceScatter; replaces the 253us tab2 AllGather):
  * After BN each core writes its local post-BN slab (s * relu(bn(h1)))
    to local DRAM (tab2loc) -- no collective.
  * Edges are partitioned by SRC core.  Each core gathers its outgoing
    edges' messages from tab2loc in chunks of 128 (dst-sorted), and the PE
    accumulates them FEATURE-MAJOR into per-(dst core, 4-block group) PSUM
    tiles via per-chunk 0/1 assignment matmuls:
        psumT[f, dst] += msg_chunk[e, f]^T  @  M_chunk[e, dst_window]
    (lhsT = the gathered chunk, rhs = a host-built 0/1 matrix).  PE sums
    are deterministic -- dma_scatter_add would race on duplicate dst rows
    on real hardware (verified experimentally).
  * Self-loop term rides along as synthetic (j -> j) edges.
  * Partial slabs go to a [8*128, npc] bf16 accumulator; ONE bf16
    ReduceScatter (priced by OUT size = 1.6MB -> ~57us vs 253us AllGather)
    hands each core its final aggregated feature-major slab.
  * Epilogue: scale by s (column-wise, feature-major), then lhsT IS already
    transposed for the Wcat matmul -- no per-block PE transposes needed.

Static SPMD choreography: chunk counts, matmul windows, and M offsets are
max-over-cores static geometry; all per-core variation lives in tensor
contents (gather idx, M values), with pad positions pointing at row 0 and
zero M columns.

Gather calls carry <=896 idxs (hw SWDGE descriptor ring holds 128 in-flight
entries per engine; larger calls hang the device).  Calls rotate over 4
SWDGE queues; queue_num is re-derived post-compile from the Tile-assigned
DMASW sem lane.
"""

import numpy as np

N_NODES = 50000
N_EDGES = 800000
D_IN = 128
D_HID = 128
D_LAT = 64
BN_EPS = 1e-5
N_CORES = 8
P = 128
LO_CORES = 5  # cores 0..4 form the "lo" table half; 5*6272=31360 < 32768
              # (dma_gather int16 indices address at most 32768 rows per call)

CALL_COLS = 7      # gather call size: 7 cols * 128 = 896 idxs (hw ring cap)
NUM_Q = 4          # SWDGE queues
GB = 4             # dst blocks per PSUM group (layer 2 push)

_CACHE = {}


# ----------------------------------------------------------------------------
# Host-side preprocessing
# ----------------------------------------------------------------------------


def _wrap_idx(lin):
    """dma_gather idx layout: position i -> [i%16, i//16], replicated to 128
    partitions. lin: [n] int array (n % 16 == 0) -> [128, n//16] int16."""
    n = lin.shape[0]
    w = lin.reshape(n // 16, 16).T.astype(np.int16)  # [16, n//16]
    return np.tile(w, (8, 1))


def _pack_calls(D, call_cols):
    """Slice the global column space into calls of <= call_cols columns.

    A call may cover partial blocks; each call carries its piece list
    [(block, col_off_in_call, width, first, last)].
    """
    C0 = np.concatenate([[0], np.cumsum(D)]).astype(np.int64)
    ct = int(C0[-1])
    calls = []
    for c0 in range(0, ct, call_cols):
        c1 = min(c0 + call_cols, ct)
        pieces = []
        for b in range(len(D)):
            lo = max(c0, int(C0[b]))
            hi = min(c1, int(C0[b + 1]))
            if lo < hi:
                pieces.append(
                    (b, lo - c0, hi - lo, lo == int(C0[b]), hi == int(C0[b + 1]))
                )
        calls.append((c0, c1 - c0, tuple(pieces)))
    return C0, calls


def _build_pass(tcoord_src, tkey_dst, n_cores, npc, blocks, call_cols,
                pad_idx, idx_base):
    """Build one gather pass layout (layer-1 pull).

    tcoord_src: per-edge source table coord (already offset for hi pass)
    tkey_dst:   per-edge dst node key in THIS pass's permutation
    Returns D [blocks], C0, calls, idx arrays [n_cores, 128, c_total] int32.
    """
    deg = np.bincount(tkey_dst, minlength=n_cores * npc)
    d3 = deg.reshape(n_cores, blocks, P)
    D = d3.max(axis=(0, 2)).astype(np.int64)
    D = np.maximum(D, 1)
    C0, calls = _pack_calls(D, call_cols)
    c_total = int(C0[-1])

    idx = np.full((n_cores, P, c_total), pad_idx - idx_base, dtype=np.int32)
    eorder = np.argsort(tkey_dst, kind="stable")
    k_s = tkey_dst[eorder]
    src_s = (tcoord_src[eorder] - idx_base).astype(np.int32)
    grp = np.searchsorted(k_s, k_s)
    slot = np.arange(k_s.size) - grp
    core_e = k_s // npc
    local_e = k_s % npc
    b_e = local_e // P
    p_e = local_e % P
    col_e = C0[b_e] + slot
    assert (slot < D[b_e]).all()
    idx[core_e, p_e, col_e] = src_s
    return D, C0, calls, idx, c_total


def _idx_to_wrapped(idx):
    """[n_cores, 128, c_total] int32 -> wrapped int16 [n_cores, 128, 8*c_total].

    Global linear position order is column-major (i = c*128 + p); contiguous
    position chunks map to contiguous wrapped columns, so any call covering
    cols [c0, c1) reads the wrapped slice [:, 8*c0 : 8*c1]."""
    n_cores, _, c_total = idx.shape
    out = np.empty((n_cores, 128, 8 * c_total), dtype=np.int16)
    for k in range(n_cores):
        lin = idx[k].T.reshape(-1)
        out[k] = _wrap_idx(lin)
    return out


def _plan_l2(src, dst, core_of, local_of, l2col_of, node_of, npc, blocks,
             s_l2):
    """Layer-2 push plan: per-src-core edge streams, static chunk/window
    choreography, per-core gather idx + 0/1 M matrices.

    The dst COLUMN order is a separate per-core permutation (l2col_of,
    sorted by total degree) -- balanced across src cores, unlike the main
    dlo-sorted layout, so the max-over-cores chunk envelope stays tight.

    Static geometry (identical across cores, SPMD):
      groups: (kk, g, g0, W, C, m_off, m_cols, windows=((w0, w1, mo), ...))
    Per-core data: idx stream (int, gather rows into tab2loc), M [128, Mtot].
    """
    n_cores = N_CORES
    GR = (blocks + GB - 1) // GB
    group_w = [min(GB * P, npc - g * GB * P) for g in range(GR)]

    # per-core edge lists sorted by (dst_core, dst_l2pos).  Self loops are
    # NOT included here -- they would land only on the own-core stream and
    # blow up the max-over-cores static envelope by ~n_local per dst core;
    # the self term is added in the epilogue from a transposed local gather
    # that runs during the ReduceScatter wait.
    ecore = []
    for k in range(n_cores):
        m = core_of[src] == k
        s_loc = local_of[src[m]]
        d_core = core_of[dst[m]]
        d_loc = l2col_of[dst[m]]
        key = d_core * npc + d_loc
        o = np.argsort(key, kind="stable")
        ecore.append((s_loc[o], d_core[o], d_loc[o], key[o]))

    # group slices per core: searchsorted bounds on key
    # chunk counts per (kk, g): ceil(max_core n / 128)
    groups = []
    per_core_chunks = [[] for _ in range(n_cores)]  # list of (idx128, jrel128)
    m_off = 0
    order_kg = [(kk, g) for kk in range(n_cores) for g in range(GR)]
    for kk, g in order_kg:
        if True:
            g0 = g * GB * P
            W = group_w[g]
            lo_key = kk * npc + g0
            hi_key = kk * npc + g0 + W
            segs = []
            for k in range(n_cores):
                keys = ecore[k][3]
                a = np.searchsorted(keys, lo_key)
                b = np.searchsorted(keys, hi_key)
                segs.append((a, b))
            nmax = max(b - a for a, b in segs)
            C = (nmax + P - 1) // P
            windows = []
            g_m0 = m_off
            for c in range(C):
                w0, w1 = W, 0
                for k in range(n_cores):
                    a, b = segs[k]
                    r0, r1 = a + c * P, min(a + (c + 1) * P, b)
                    if r0 < r1:
                        j = ecore[k][2][r0:r1] - g0
                        w0 = min(w0, int(j.min()))
                        w1 = max(w1, int(j.max()) + 1)
                if w1 <= w0:
                    w0, w1 = 0, 1
                windows.append((w0, w1, m_off - g_m0))
                m_off += w1 - w0
            groups.append((kk, g, g0, W, C, g_m0, m_off - g_m0,
                           tuple(windows)))
            for k in range(n_cores):
                a, b = segs[k]
                for c in range(C):
                    r0, r1 = a + c * P, min(a + (c + 1) * P, b)
                    n = max(0, r1 - r0)
                    idx128 = np.zeros(P, np.int32)
                    jrel = np.full(P, -1, np.int32)
                    if n > 0:
                        sl = ecore[k][0][r0:r1]
                        idx128[:n] = (sl % P) * blocks + sl // P
                        jrel[:n] = ecore[k][2][r0:r1] - g0
                    per_core_chunks[k].append((idx128, jrel))

    total_chunks = sum(gr[4] for gr in groups)
    m_total = m_off
    n_calls = (total_chunks + CALL_COLS - 1) // CALL_COLS

    # per-core tensors
    widx2 = []
    Ms = []
    for k in range(n_cores):
        lin = np.concatenate([c[0] for c in per_core_chunks[k]])
        widx2.append(_wrap_idx(lin))
        M = np.zeros((P, m_total), np.float32)
        t = 0
        for kk, g, g0, W, C, g_m0, g_mc, windows in groups:
            for c in range(C):
                idx128, jrel = per_core_chunks[k][t]
                w0, w1, mo = windows[c]
                rows = np.nonzero(jrel >= 0)[0]
                # M carries the dst-side s factor (out = s * sum), so the
                # epilogue needs no post-matmul scaling at all.
                np.add.at(M, (rows, g_m0 + mo + (jrel[rows] - w0)),
                          s_l2[kk][g0 + jrel[rows]])
                t += 1
        Ms.append(M)

    max_mw = max((gr[6] for gr in groups), default=1)
    max_win = max((w1 - w0 for gr in groups for (w0, w1, _) in gr[7]),
                  default=1)
    return dict(
        groups=tuple(groups),
        total_chunks=total_chunks,
        m_total=m_total,
        n_calls=n_calls,
        max_mw=max_mw,
        max_win=max_win,
        widx2=widx2,
        Ms=Ms,
    )


def _plan(edge_index, n_nodes, n_cores, call_cols):
    src = np.asarray(edge_index[0], dtype=np.int64)
    dst = np.asarray(edge_index[1], dtype=np.int64)

    deg_in = np.bincount(dst, minlength=n_nodes).astype(np.int64)
    s = (1.0 / np.sqrt((deg_in + 1).astype(np.float64))).astype(np.float32)

    n_local = (n_nodes + n_cores - 1) // n_cores
    blocks = (n_local + 1 + P - 1) // P
    npc = blocks * P
    lo_rows = LO_CORES * npc
    assert lo_rows < 32768 and (n_cores * npc - lo_rows) < 32768

    # ---- core assignment: deal by total-degree rank (balances edge load and
    # aligns block-degree profiles across cores).
    order = np.argsort(-deg_in, kind="stable")
    rank_of = np.empty(n_nodes, dtype=np.int64)
    rank_of[order] = np.arange(n_nodes)
    core_of = rank_of % n_cores

    src_is_lo = core_of[src] < LO_CORES
    dlo = np.bincount(dst[src_is_lo], minlength=n_nodes)
    dhi = np.bincount(dst[~src_is_lo], minlength=n_nodes)

    # main layout: per-core locals sorted by lo-degree (tight LO padding)
    local_of = np.empty(n_nodes, dtype=np.int64)
    node2hi = np.empty(n_nodes, dtype=np.int64)
    for k in range(n_cores):
        nodes_k = np.nonzero(core_of == k)[0]
        o = nodes_k[np.argsort(-dlo[nodes_k], kind="stable")]
        local_of[o] = np.arange(o.size)
        o2 = nodes_k[np.argsort(-dhi[nodes_k], kind="stable")]
        node2hi[o2] = k * npc + np.arange(o2.size)
    node2table = core_of * npc + local_of

    # ---- gather-source row numbering: tables are [(core,part), (block,feat)]
    # 2-D tensors, so node (core k, local j=b*128+p) lives at flat row
    # (k*128+p)*blocks + b of its half (hi half: k-LO_CORES).
    def kpb_row(core, local, core0):
        return ((core - core0) * P + local % P) * blocks + local // P

    node2row = np.where(
        core_of < LO_CORES,
        kpb_row(core_of, local_of, 0),
        kpb_row(core_of, local_of, LO_CORES),
    )
    pad_lo = kpb_row(0, npc - 1, 0)
    pad_hi = kpb_row(N_CORES - 1, npc - 1, LO_CORES)

    # ---- LO pass on the main permutation
    D_lo, C0_lo, calls_lo, idx_lo, ct_lo = _build_pass(
        node2row[src[src_is_lo]], node2table[dst[src_is_lo]], n_cores, npc,
        blocks, call_cols, pad_lo, 0,
    )

    # ---- HI pass on the hi permutation
    D_hi, C0_hi, calls_hi, idx_hi, ct_hi = _build_pass(
        node2row[src[~src_is_lo]], node2hi[dst[~src_is_lo]], n_cores, npc,
        blocks, call_cols, pad_hi, 0,
    )

    # ---- combine map: main-layout local j gets acc_hi[himap[j]] added
    himap = np.full((n_cores, npc), npc - 1, dtype=np.int64)  # pad -> pad row
    for k in range(n_cores):
        nodes_k = np.nonzero(core_of == k)[0]
        himap[k, local_of[nodes_k]] = node2hi[nodes_k] % npc

    widx_lo = _idx_to_wrapped(idx_lo)
    widx_hi = _idx_to_wrapped(idx_hi)
    himap_row = (himap % P) * blocks + himap // P
    widx_cb = np.stack([_wrap_idx(himap_row[k]) for k in range(n_cores)])

    # per-core node lists and s in the MAIN layout
    node_of = np.full((n_cores, npc), -1, dtype=np.int64)
    s_arr = np.zeros((n_cores, P, blocks), dtype=np.float32)
    for k in range(n_cores):
        nodes_k = np.nonzero(core_of == k)[0]
        loc = local_of[nodes_k]
        node_of[k, loc] = nodes_k
        s_arr[k, loc % P, loc // P] = s[nodes_k]

    # layer-2 dst column permutation: per-core total-degree sort (balanced
    # per-src-core edge counts -> tight static chunk envelope)
    l2col_of = np.empty(n_nodes, dtype=np.int64)
    node_of_l2 = np.full((n_cores, npc), -1, dtype=np.int64)
    s_l2 = np.zeros((n_cores, npc), dtype=np.float32)
    wself = []
    for k in range(n_cores):
        nodes_k = np.nonzero(core_of == k)[0]
        o = nodes_k[np.argsort(-deg_in[nodes_k], kind="stable")]
        l2col_of[o] = np.arange(o.size)
        node_of_l2[k, : o.size] = o
        s_l2[k, : o.size] = s[o]
        rows = np.zeros(npc, dtype=np.int64)
        loc = local_of[o]
        rows[: o.size] = (loc % P) * blocks + loc // P
        wself.append(_wrap_idx(rows))

    l2 = _plan_l2(src, dst, core_of, local_of, l2col_of, node_of, npc, blocks,
                  s_l2)

    return dict(
        s=s,
        node2table=node2table,
        node_of=node_of,
        npc=npc,
        blocks=blocks,
        n_local=n_local,
        lo_rows=lo_rows,
        s_arr=s_arr,
        D_lo=D_lo, C0_lo=C0_lo, calls_lo=calls_lo, idx_lo=idx_lo, ct_lo=ct_lo,
        D_hi=D_hi, C0_hi=C0_hi, calls_hi=calls_hi, idx_hi=idx_hi, ct_hi=ct_hi,
        himap=himap,
        widx_lo=widx_lo, widx_hi=widx_hi, widx_cb=widx_cb,
        node_of_l2=node_of_l2, s_l2=s_l2, wself=wself,
        l2=l2,
    )


def _to_bf16(a):
    import ml_dtypes

    return a.astype(ml_dtypes.bfloat16)


def _host_inputs(plan, x, W1, Wmu, Wls, bmu, bls, gamma, beta):
    npc = plan["npc"]
    node_of = plan["node_of"]
    s = plan["s"]
    l2 = plan["l2"]
    wcat = np.concatenate([Wmu, Wls], axis=1).astype(np.float32)
    bcat = np.concatenate([bmu, bls]).astype(np.float32).reshape(D_HID, 1)

    # s-prescaled x in the partition-major table layout:
    # row (core*128 + p), col (block*128 + f) holds node (core, b*128+p).
    blocks = npc // P
    xs_tab = np.zeros((N_CORES * npc, x.shape[1]), dtype=np.float32)
    for k in range(N_CORES):
        nodes = node_of[k]
        valid = nodes >= 0
        xs_tab[k * npc + np.nonzero(valid)[0]] = (
            x[nodes[valid]] * s[nodes[valid]][:, None]
        )
    xs_kpb = (
        xs_tab.reshape(N_CORES, blocks, P, D_IN)
        .transpose(0, 2, 1, 3)
        .reshape(N_CORES * P, blocks * D_IN)
    )
    xs_kpb = _to_bf16(xs_kpb)
    xs_lo = np.ascontiguousarray(xs_kpb[: LO_CORES * P])
    xs_hi = np.ascontiguousarray(xs_kpb[LO_CORES * P :])

    per_core = []
    for k in range(N_CORES):
        sa = plan["s_arr"][k]  # [P, blocks]
        # layer-2 epilogue scale: s per output partition (L2 column order:
        # block b, partition p <-> l2 position b*128+p)
        scol2 = np.ascontiguousarray(
            plan["s_l2"][k].reshape(blocks, P).T.astype(np.float32))
        per_core.append(
            {
                "xslo": xs_lo,
                "xshi": xs_hi,
                "xso": np.ascontiguousarray(xs_kpb[k * P : (k + 1) * P]),
                "W1": np.ascontiguousarray(_to_bf16(W1)),
                "Wcat": np.ascontiguousarray(_to_bf16(wcat)),
                "bcat": bcat,
                "s_arr": np.ascontiguousarray(sa),
                "widx_lo": np.ascontiguousarray(plan["widx_lo"][k]),
                "widx_hi": np.ascontiguousarray(plan["widx_hi"][k]),
                "widx_cb": np.ascontiguousarray(plan["widx_cb"][k]),
                "gamma": gamma.astype(np.float32).reshape(D_HID, 1),
                "beta": beta.astype(np.float32).reshape(D_HID, 1),
                "widx2": np.ascontiguousarray(l2["widx2"][k]),
                "M2": np.ascontiguousarray(_to_bf16(l2["Ms"][k])),
                "scol2": scol2,
                "wself": np.ascontiguousarray(plan["wself"][k]),
            }
        )
    return per_core


def _postprocess(plan, outs):
    n_nodes = int(plan["node_of"].max()) + 1
    mu = np.zeros((n_nodes, D_LAT), dtype=np.float32)
    ls = np.zeros((n_nodes, D_LAT), dtype=np.float32)
    node_of = plan["node_of_l2"]  # out_cat columns are in the L2 permutation
    npc = node_of.shape[1]
    for k in range(N_CORES):
        nodes = node_of[k]
        valid = nodes >= 0
        o = np.asarray(outs[k]).astype(np.float32)  # [128 outf, npc] f-major
        cols = valid.nonzero()[0]
        mu[nodes[valid]] = o[:D_LAT, cols].T
        ls[nodes[valid]] = o[D_LAT:, cols].T
    return mu, ls


# ----------------------------------------------------------------------------
# Device program
# ----------------------------------------------------------------------------


def _build_program(geom):
    from concourse import bacc, bass, mybir, tile
    from concourse.masks import make_identity

    (npc, blocks, D_lo, calls_lo, ct_lo, D_hi, calls_hi, ct_hi, n_real,
     call_cols, lo_rows, l2g) = geom
    (groups, total_chunks, m_total, n_calls, max_mw, max_win) = l2g
    D_lo, D_hi = list(D_lo), list(D_hi)
    f32 = mybir.dt.float32
    bf16 = mybir.dt.bfloat16
    i16 = mybir.dt.int16
    GR = (blocks + GB - 1) // GB

    nc = bacc.Bacc("TRN2", target_bir_lowering=False, debug=False,
                   num_devices=N_CORES, num_swdge_queues=NUM_Q)

    t_xslo = nc.dram_tensor("xslo", [LO_CORES * P, npc], bf16,
                            kind="ExternalInput")
    t_xshi = nc.dram_tensor("xshi", [(N_CORES - LO_CORES) * P, npc], bf16,
                            kind="ExternalInput")
    t_xso = nc.dram_tensor("xso", [P, npc], bf16, kind="ExternalInput")
    t_W1 = nc.dram_tensor("W1", [P, D_HID], bf16, kind="ExternalInput")
    t_Wcat = nc.dram_tensor("Wcat", [D_HID, P], bf16, kind="ExternalInput")
    t_bcat = nc.dram_tensor("bcat", [P, 1], f32, kind="ExternalInput")
    t_sarr = nc.dram_tensor("s_arr", [P, blocks], f32, kind="ExternalInput")
    t_wlo = nc.dram_tensor("widx_lo", [P, 8 * ct_lo], i16, kind="ExternalInput")
    t_whi = nc.dram_tensor("widx_hi", [P, 8 * ct_hi], i16, kind="ExternalInput")
    t_wcb = nc.dram_tensor("widx_cb", [P, npc // 16], i16, kind="ExternalInput")
    t_gamma = nc.dram_tensor("gamma", [D_HID, 1], f32, kind="ExternalInput")
    t_beta = nc.dram_tensor("beta", [D_HID, 1], f32, kind="ExternalInput")
    t_widx2 = nc.dram_tensor("widx2", [P, total_chunks * P // 16], i16,
                             kind="ExternalInput")
    t_M2 = nc.dram_tensor("M2", [P, m_total], bf16, kind="ExternalInput")
    t_scol2 = nc.dram_tensor("scol2", [P, blocks], f32, kind="ExternalInput")
    t_wself = nc.dram_tensor("wself", [P, npc // 16], i16,
                             kind="ExternalInput")
    t_out = nc.dram_tensor("out_cat", [P, npc], bf16, kind="ExternalOutput")

    tab2loc = nc.dram_tensor("tab2loc", [npc, P], bf16)
    acc1 = nc.dram_tensor("acc_hi1", [P, npc], bf16)
    accL2 = nc.dram_tensor("accL2", [N_CORES * P, npc], bf16)
    rs_out = nc.dram_tensor("rs_out", [P, npc], bf16)
    st_in = nc.dram_tensor("st_in", [2 * D_HID], f32)
    st_out = nc.dram_tensor("st_out", [N_CORES * 2 * D_HID], f32,
                            addr_space="Shared")

    groups_rep = [list(range(N_CORES))]
    inv_n = 1.0 / float(n_real)

    with tile.TileContext(nc) as tc:
        with (
            tc.tile_pool(name="persist", bufs=1) as persist,
            tc.tile_pool(name="stream", bufs=4) as stream,
            tc.tile_pool(name="gath", bufs=3) as gath,
            tc.tile_pool(name="gath2", bufs=4) as gath2,
            tc.tile_pool(name="mpool", bufs=3) as mpool,
            tc.tile_pool(name="ppool", bufs=3) as ppool,
            tc.tile_pool(name="gathcb", bufs=1) as gathcb,
            tc.tile_pool(name="small", bufs=1) as small,
            tc.tile_pool(name="ps", bufs=2, space="PSUM") as psp,
            tc.tile_pool(name="ps_blk", bufs=3, space="PSUM") as psblk,
            tc.tile_pool(name="ps_acc", bufs=1, space="PSUM") as psacc,
        ):
            H = persist.tile([P, npc], bf16, tag="H")
            ownslab = persist.tile([P, npc], bf16, tag="ownslab")
            accsb = persist.tile([P, npc], bf16, tag="accsb")
            outsb = persist.tile([P, npc], bf16, tag="outsb")
            wlo = persist.tile([P, 8 * ct_lo], i16, tag="wlo")
            whi = persist.tile([P, 8 * ct_hi], i16, tag="whi")
            wcb = persist.tile([P, npc // 16], i16, tag="wcb")
            widx2 = persist.tile([P, total_chunks * P // 16], i16,
                                 tag="widx2")
            wself = persist.tile([P, npc // 16], i16, tag="wself")
            selfT = persist.tile([P, npc], bf16, tag="selfT")
            w1 = small.tile([P, D_HID], bf16, tag="w1")
            wcat = small.tile([D_HID, P], bf16, tag="wcat")
            sarr = small.tile([P, blocks], f32, tag="sarr")
            scol2 = small.tile([P, blocks], f32, tag="scol2")
            gcol = small.tile([D_HID, 1], f32, tag="gcol")
            bcol = small.tile([D_HID, 1], f32, tag="bcol")
            bccol = small.tile([P, 1], f32, tag="bccol")
            ident = small.tile([P, P], bf16, tag="ident")
            identf = small.tile([P, P], f32, tag="identf")
            zeroM = small.tile([P, GB * P], bf16, tag="zeroM")
            ones_col = small.tile([P, 1], bf16, tag="ones_col")
            ones_row = small.tile([1, P], f32, tag="ones_row")
            grep = small.tile([P, P], f32, tag="grep")
            brep = small.tile([P, P], f32, tag="brep")
            grep4 = small.tile([P, 4 * P], bf16, tag="grep4")
            brep4 = small.tile([P, 4 * P], bf16, tag="brep4")
            bcrep = small.tile([P, P], f32, tag="bcrep")
            bcrep4 = small.tile([P, 4 * P], f32, tag="bcrep4")

            nc.sync.dma_start(out=whi[:], in_=t_whi[:])
            nc.sync.dma_start(out=wlo[:], in_=t_wlo[:])
            nc.sync.dma_start(out=wcb[:], in_=t_wcb[:])
            nc.sync.dma_start(out=widx2[:], in_=t_widx2[:])
            nc.sync.dma_start(out=wself[:], in_=t_wself[:])
            nc.sync.dma_start(out=scol2[:], in_=t_scol2[:])
            nc.sync.dma_start(out=w1[:], in_=t_W1[:])
            nc.sync.dma_start(out=wcat[:], in_=t_Wcat[:])
            nc.sync.dma_start(out=sarr[:], in_=t_sarr[:])
            nc.sync.dma_start(out=gcol[:], in_=t_gamma[:])
            nc.sync.dma_start(out=bcol[:], in_=t_beta[:])
            nc.sync.dma_start(out=bccol[:], in_=t_bcat[:])
            make_identity(nc, ident[:])
            make_identity(nc, identf[:])
            nc.vector.memset(zeroM[:], 0.0)
            nc.vector.memset(ones_col[:], 1.0)
            nc.vector.memset(ones_row[:], 1.0)
            warm = small.tile([1, 1], f32, tag="warm")
            nc.vector.memset(warm[:], 1.0)
            nc.scalar.activation(out=warm[:], in_=warm[:],
                                 func=mybir.ActivationFunctionType.Sqrt)

            def outer_bcast(col_ap, dst_tile):
                pst = psp.tile([P, P], f32, space="PSUM", tag="ps_big")
                nc.tensor.transpose(out=pst[0:1, :], in_=col_ap,
                                    identity=identf[:])
                row = stream.tile([1, P], f32, tag="rowbuf")
                nc.vector.tensor_copy(out=row[:], in_=pst[0:1, :])
                psb = psp.tile([P, P], f32, space="PSUM", tag="ps_big")
                nc.tensor.matmul(out=psb[:], lhsT=ones_row[:], rhs=row[:],
                                 start=True, stop=True)
                nc.vector.tensor_copy(out=dst_tile[:], in_=psb[:])

            outer_bcast(bccol[:], bcrep)
            for r in range(4):
                nc.vector.tensor_copy(out=bcrep4[:, r * P : (r + 1) * P],
                                      in_=bcrep[:])

            # --- layer 1 gathers raw (s*x): A(x@W1) == A(x)@W1, so W1 is
            # applied per-block AFTER aggregation; the host-marshaled xslo/
            # xshi inputs ARE the gather tables (no stage-1, no table write).
            nc.sync.dma_start(out=ownslab[:], in_=t_xso[:, :])

            ps_stats = psacc.tile([D_HID, 2], f32, space="PSUM",
                                  tag="ps_stats")

            qn = [0]

            def next_q():
                q = qn[0]
                qn[0] = (qn[0] + 1) % NUM_Q
                return q

            def gather(out_ap, table_ap, widx_ap, n_idx):
                nc.gpsimd.dma_gather(
                    out_ap.rearrange("p (c f) -> p c f", f=P),
                    table_ap,
                    widx_ap,
                    num_idxs=n_idx,
                    num_idxs_reg=n_idx,
                    elem_size=P,
                    queue_num=next_q(),
                )

            def aggregate(tlo, thi, acc_t, selfslab):
                """Layer-1 pull aggregation -> H blocks + BN stats."""
                grp = {}

                def blk_view(b, pre):
                    g, r = b // GB, b % GB
                    if g not in grp:
                        grp[g] = psblk.tile([P, GB * P], f32, space="PSUM",
                                            tag="ps_blk",
                                            name=f"{pre}_{g}")
                    return grp[g][:, r * P : (r + 1) * P]

                g_open = set()

                def g_last(b):
                    return min((b // GB) * GB + GB - 1, blocks - 1)

                def flags(b, is_first_mm, is_last_mm):
                    g = b // GB
                    start = is_first_mm and g not in g_open
                    if start:
                        g_open.add(g)
                    stop = is_last_mm and b == g_last(b)
                    return start, stop

                for c0, cols, pieces in calls_hi:
                    gt = gath.tile([P, call_cols * P], bf16, tag="gt")
                    gather(gt[:, : cols * P],
                           thi.rearrange("q (r f) -> (q r) f", f=P),
                           whi[:, 8 * c0 : 8 * (c0 + cols)], cols * P)
                    for b, o, d, first, last in pieces:
                        bv = blk_view(b, "psg_hi")
                        for i in range(d):
                            st_, sp_ = flags(b, first and i == 0,
                                             last and i == d - 1)
                            nc.tensor.matmul(
                                out=bv, lhsT=ident[:],
                                rhs=gt[:, (o + i) * P : (o + i + 1) * P],
                                start=st_, stop=sp_)
                        if last and b == g_last(b):
                            g = b // GB
                            w = (b % GB) + 1
                            nc.vector.tensor_copy(
                                out=accsb[:, g * GB * P : g * GB * P + w * P],
                                in_=grp.pop(g)[:, : w * P])
                nc.sync.dma_start(out=acc_t[:, :], in_=accsb[:])

                # combine gathers (cbt[:, j] = acc_t[himap[j]]) are issued
                # a few calls INTO the lo stream so the acc-write latency
                # hides behind lo gather work.
                cbt = gathcb.tile([P, npc], bf16, tag="gtcb")

                grp.clear()
                g_open.clear()

                def start_block(b):
                    bv = blk_view(b, "psg_lo")
                    st_, _ = flags(b, True, False)
                    nc.tensor.matmul(out=bv, lhsT=ident[:],
                                     rhs=selfslab[:, b * P : (b + 1) * P],
                                     start=st_, stop=False)
                    return bv

                cb_at = min(1, (D_lo[0] - 1) // call_cols)
                for ci, (c0, cols, pieces) in enumerate(calls_lo):
                    if ci == cb_at:
                        for cb0 in range(0, blocks, call_cols):
                            cb1 = min(cb0 + call_cols, blocks)
                            gather(cbt[:, cb0 * P : cb1 * P],
                                   acc_t.rearrange("q (r f) -> (q r) f", f=P),
                                   wcb[:, 8 * cb0 : 8 * cb1],
                                   (cb1 - cb0) * P)
                    gt = gath.tile([P, call_cols * P], bf16, tag="gt")
                    gather(gt[:, : cols * P],
                           tlo.rearrange("q (r f) -> (q r) f", f=P),
                           wlo[:, 8 * c0 : 8 * (c0 + cols)], cols * P)
                    for b, o, d, first, last in pieces:
                        if first:
                            start_block(b)
                        bv = blk_view(b, "psg_lo")
                        for i in range(d):
                            nc.tensor.matmul(
                                out=bv, lhsT=ident[:],
                                rhs=gt[:, (o + i) * P : (o + i + 1) * P],
                                start=False, stop=False)
                        if last:
                            _, sp_ = flags(b, False, True)
                            nc.tensor.matmul(
                                out=bv, lhsT=ident[:],
                                rhs=cbt[:, b * P : (b + 1) * P],
                                start=False, stop=sp_)
                        if last and b == g_last(b):
                            g = b // GB
                            gt_ps = grp.pop(g)
                            for r in range(b % GB + 1):
                                bb = g * GB + r
                                sl = slice(bb * P, (bb + 1) * P)
                                # U = s*(agg); h1 = U @ W1
                                ub = stream.tile([P, P], bf16, tag="ub")
                                nc.vector.tensor_scalar_mul(
                                    ub[:], gt_ps[:, r * P : (r + 1) * P],
                                    sarr[:, bb : bb + 1])
                                psT = psp.tile([P, P], bf16, space="PSUM",
                                               tag="ps_bigT")
                                nc.tensor.transpose(out=psT[:], in_=ub[:],
                                                    identity=ident[:])
                                uT = stream.tile([P, P], bf16, tag="uT")
                                nc.vector.tensor_copy(out=uT[:], in_=psT[:])
                                ps1 = psp.tile([P, P], f32, space="PSUM",
                                               tag="ps_big")
                                nc.tensor.matmul(out=ps1[:], lhsT=uT[:],
                                                 rhs=w1[:], start=True,
                                                 stop=True)
                                nc.vector.tensor_copy(out=H[:, sl],
                                                      in_=ps1[:])
                                sq = stream.tile([P, D_HID], bf16,
                                                 tag="sq")
                                nc.scalar.square(out=sq[:], in_=H[:, sl])
                                nc.tensor.matmul(
                                    out=ps_stats[:, 0:1], lhsT=H[:, sl],
                                    rhs=ones_col[:],
                                    start=(bb == 0), stop=False)
                                nc.tensor.matmul(
                                    out=ps_stats[:, 1:2], lhsT=sq[:],
                                    rhs=ones_col[:],
                                    start=False,
                                    stop=(bb == blocks - 1))

            aggregate(t_xslo, t_xshi, acc1, ownslab)

            # --- BN ---------------------------------------------------------
            st = small.tile([D_HID, 2], f32, tag="st")
            nc.vector.tensor_copy(out=st[:], in_=ps_stats[:])
            nc.sync.dma_start(out=st_in[:], in_=st[:])
            nc.gpsimd.collective_compute(
                "AllGather", mybir.AluOpType.bypass,
                replica_groups=groups_rep,
                ins=[st_in[:]], outs=[st_out[:]],
            )
            st8 = small.tile([D_HID, 2 * N_CORES], f32, tag="st8")
            nc.sync.dma_start(
                out=st8.rearrange("p (c t) -> p c t", t=2),
                in_=st_out[:].rearrange("(c p t) -> p c t", p=D_HID, t=2),
            )
            st2 = small.tile([D_HID, 2], f32, tag="st2")
            nc.vector.reduce_sum(
                out=st2[:],
                in_=st8.rearrange("p (c t) -> p t c", t=2),
                axis=mybir.AxisListType.X)

            eps_col = small.tile([D_HID, 1], f32, tag="eps_col")
            nc.vector.memset(eps_col[:], BN_EPS)
            mean = small.tile([D_HID, 1], f32, tag="mean")
            msq = small.tile([D_HID, 1], f32, tag="msq")
            var = small.tile([D_HID, 1], f32, tag="var")
            std = small.tile([D_HID, 1], f32, tag="std")
            istd = small.tile([D_HID, 1], f32, tag="istd")
            gp = small.tile([D_HID, 1], f32, tag="gp")
            bp_ = small.tile([D_HID, 1], f32, tag="bp")
            nc.vector.tensor_scalar_mul(mean[:], st2[:, 0:1], inv_n)
            nc.vector.tensor_scalar_mul(msq[:], st2[:, 1:2], inv_n)
            nc.scalar.square(out=var[:], in_=mean[:])
            nc.vector.tensor_tensor(out=var[:], in0=msq[:], in1=var[:],
                                    op=mybir.AluOpType.subtract)
            nc.scalar.activation(out=std[:], in_=var[:],
                                 func=mybir.ActivationFunctionType.Sqrt,
                                 bias=eps_col[:])
            nc.vector.reciprocal(out=istd[:], in_=std[:])
            nc.vector.tensor_tensor(out=gp[:], in0=gcol[:], in1=istd[:],
                                    op=mybir.AluOpType.mult)
            nc.vector.tensor_tensor(out=bp_[:], in0=mean[:], in1=gp[:],
                                    op=mybir.AluOpType.mult)
            nc.vector.tensor_tensor(out=bp_[:], in0=bcol[:], in1=bp_[:],
                                    op=mybir.AluOpType.subtract)
            outer_bcast(gp[:], grep)
            outer_bcast(bp_[:], brep)
            for r in range(4):
                nc.vector.tensor_copy(out=grep4[:, r * P : (r + 1) * P],
                                      in_=grep[:])
                nc.vector.tensor_copy(out=brep4[:, r * P : (r + 1) * P],
                                      in_=brep[:])

            # BN affine + ReLU + s-scale, written to local DRAM table
            # (tab2loc rows = p*blocks + b -- the layer-2 gather source).
            t2v = tab2loc[:, :].rearrange("(p b) f -> p b f", b=blocks)
            for b0 in range(0, blocks, 4):
                b1 = min(b0 + 4, blocks)
                w = (b1 - b0) * P
                t1 = stream.tile([P, 4 * P], bf16, tag="bn1")
                nc.vector.tensor_tensor(out=t1[:, :w], in0=H[:, b0 * P:b0 * P + w],
                                        in1=grep4[:, :w],
                                        op=mybir.AluOpType.mult)
                nc.vector.tensor_tensor(out=t1[:, :w], in0=t1[:, :w],
                                        in1=brep4[:, :w],
                                        op=mybir.AluOpType.add)
                nc.scalar.activation(out=t1[:, :w], in_=t1[:, :w],
                                     func=mybir.ActivationFunctionType.Relu)
                t2 = stream.tile([P, 4 * P], bf16, tag="bn2")
                for b in range(b0, b1):
                    r = b - b0
                    nc.vector.tensor_scalar_mul(
                        t2[:, r * P : (r + 1) * P],
                        t1[:, r * P : (r + 1) * P], sarr[:, b : b + 1])
                nc.sync.dma_start(out=t2v[:, b0:b1, :],
                                  in_=t2[:, :w].rearrange(
                                      "p (b f) -> p b f", f=P))

            # --- layer 2: push aggregation (feature-major M-matmuls) -------
            chunk_t = 0
            issued = {}

            def ensure_call(t):
                ci = t // CALL_COLS
                if ci not in issued:
                    t0 = ci * CALL_COLS
                    cols = min(CALL_COLS, total_chunks - t0)
                    gt = gath2.tile([P, CALL_COLS * P], bf16, tag="gt2")
                    gather(gt[:, : cols * P], tab2loc[:, :],
                           widx2[:, 8 * t0 : 8 * (t0 + cols)], cols * P)
                    issued[ci] = gt
                return issued[ci], t % CALL_COLS

            def emit_self():
                # self term: gather own rows in L2 column order, scale by s
                # (per-partition, node-major), transpose to feature-major.
                # Runs while the ReduceScatter holds the Pool queue.
                for c0 in range(0, blocks, CALL_COLS):
                    c1 = min(c0 + CALL_COLS, blocks)
                    sg = stream.tile([P, CALL_COLS * P], bf16, tag="sg")
                    gather(sg[:, : (c1 - c0) * P], tab2loc[:, :],
                           wself[:, 8 * c0 : 8 * c1], (c1 - c0) * P)
                    for b in range(c0, c1):
                        sgs = stream.tile([P, P], bf16, tag="sgs")
                        nc.vector.tensor_scalar_mul(
                            sgs[:], sg[:, (b - c0) * P : (b - c0 + 1) * P],
                            scol2[:, b : b + 1])
                        psT2 = psp.tile([P, P], bf16, space="PSUM",
                                        tag="ps_bigT")
                        nc.tensor.transpose(out=psT2[:], in_=sgs[:],
                                            identity=ident[:])
                        nc.vector.tensor_copy(
                            out=selfT[:, b * P : (b + 1) * P], in_=psT2[:])

            for gi, (kk, g, g0, W, C, g_m0, g_mc, windows) in enumerate(groups):
                Mt = None
                if g_mc > 0:
                    Mt = mpool.tile([P, max_mw], bf16, tag="mt")
                    nc.scalar.dma_start(out=Mt[:, :g_mc],
                                        in_=t_M2[:, g_m0 : g_m0 + g_mc])
                ps2 = psblk.tile([P, GB * P], f32, space="PSUM", tag="ps_blk",
                                 name=f"ps2_{gi}")
                nc.tensor.matmul(out=ps2[:, :W], lhsT=ident[:],
                                 rhs=zeroM[:, :W], start=True, stop=(C == 0))
                for c in range(C):
                    gt, pos = ensure_call(chunk_t)
                    w0, w1, mo = windows[c]
                    nc.tensor.matmul(
                        out=ps2[:, w0:w1],
                        lhsT=gt[:, pos * P : (pos + 1) * P],
                        rhs=Mt[:, mo : mo + (w1 - w0)],
                        start=False, stop=(c == C - 1))
                    chunk_t += 1
                pt = ppool.tile([P, GB * P], bf16, tag="pt")
                nc.vector.tensor_copy(out=pt[:, :W], in_=ps2[:, :W])
                nc.sync.dma_start(out=accL2[kk * P : (kk + 1) * P,
                                            g0 : g0 + W],
                                  in_=pt[:, :W])
            emit_self()

            nc.gpsimd.collective_compute(
                "ReduceScatter", mybir.AluOpType.add,
                replica_groups=groups_rep,
                ins=[accL2[:]], outs=[rs_out[:]],
            )

            # --- epilogue: out[outf, j] = wcat^T @ (rs + selfT) + bcat.
            # wcat is the STATIONARY operand (loaded once, no Ld churn); the
            # output is feature-major ([128 outf, npc]) and _postprocess
            # transposes on the host.  No post-scale (M/selfT carry s); bias
            # is a per-partition DVE scalar-add.  H loads ride the Act queue
            # so they don't serialize with the out stores on SP.
            def epilogue(rs_t, b0, b1, off):
                for c0 in range(b0, b1, GB):
                    c1 = min(c0 + GB, b1)
                    w = (c1 - c0) * P
                    nc.scalar.dma_start(
                        out=H[:, c0 * P : c0 * P + w],
                        in_=rs_t[:, c0 * P - off : c0 * P - off + w])
                    pse = psblk.tile([P, GB * P], f32, space="PSUM",
                                     tag="ps_blk", name=f"pse_{c0}")
                    nc.tensor.matmul(out=pse[:, :w], lhsT=wcat[:],
                                     rhs=H[:, c0 * P : c0 * P + w],
                                     start=True, stop=False)
                    nc.tensor.matmul(out=pse[:, :w], lhsT=wcat[:],
                                     rhs=selfT[:, c0 * P : c0 * P + w],
                                     start=False, stop=True)
                    nc.vector.tensor_scalar_add(
                        outsb[:, c0 * P : c0 * P + w], pse[:, :w], bccol[:])
                    nc.sync.dma_start(out=t_out[:, c0 * P : c0 * P + w],
                                      in_=outsb[:, c0 * P : c0 * P + w])

            epilogue(rs_out, 0, blocks, 0)

    nc.compile()
    if NUM_Q > 1:
        # Tile assigns SWDGE completion-sem lanes (DMASW0..7) round-robin in
        # SCHEDULED order, which differs from creation order.  Each sem lane
        # must be driven by a single SWDGE queue, so re-derive queue_num from
        # the assigned lane.
        import re as _re

        for _blk in nc.m.functions[0].blocks:
            for _inst in _blk.instructions:
                if type(_inst).__name__ == "InstDMAGatherAnt":
                    _m = _re.search(r"DMASW(\d+)_",
                                    str(_inst.sync_info.on_update[0]))
                    _inst.queue_num = int(_m.group(1)) % NUM_Q
    return nc


# ----------------------------------------------------------------------------
# Entry point
# ----------------------------------------------------------------------------

_IN_NAMES = ["xslo", "xshi", "xso", "W1", "Wcat", "bcat", "s_arr",
             "widx_lo", "widx_hi", "widx_cb", "gamma", "beta",
             "widx2", "M2", "scol2", "wself"]


def _geom(plan, call_cols):
    l2 = plan["l2"]
    l2g = (
        tuple(l2["groups"]),
        int(l2["total_chunks"]),
        int(l2["m_total"]),
        int(l2["n_calls"]),
        int(l2["max_mw"]),
        int(l2["max_win"]),
    )
    return (
        plan["npc"],
        plan["blocks"],
        tuple(int(d) for d in plan["D_lo"]),
        tuple(plan["calls_lo"]),
        plan["ct_lo"],
        tuple(int(d) for d in plan["D_hi"]),
        tuple(plan["calls_hi"]),
        plan["ct_hi"],
        int(plan["node_of"].max()) + 1,
        call_cols,
        plan["lo_rows"],
        l2g,
    )


def _run_hw(nc, per_core, trace=False, trace_cores=None):
    from concourse import bass_utils

    in_maps = [{nm: per_core[k][nm] for nm in _IN_NAMES} for k in range(N_CORES)]
    res = bass_utils.run_bass_kernel_spmd(
        nc, in_maps, core_ids=list(range(N_CORES)), trace=trace,
        trace_cores=trace_cores,
    )
    outs = [res.results[k]["out_cat"] for k in range(N_CORES)]
    return outs, res


def kernel(x, edge_index, W1, b1, gamma, beta, Wmu, bmu, Wls, bls):
    x = np.asarray(x, dtype=np.float32)
    edge_index = np.asarray(edge_index)
    W1 = np.asarray(W1, dtype=np.float32)
    gamma = np.asarray(gamma, dtype=np.float32)
    beta = np.asarray(beta, dtype=np.float32)
    Wmu = np.asarray(Wmu, dtype=np.float32)
    bmu = np.asarray(bmu, dtype=np.float32)
    Wls = np.asarray(Wls, dtype=np.float32)
    bls = np.asarray(bls, dtype=np.float32)

    plan = _plan(edge_index, x.shape[0], N_CORES, call_cols=CALL_COLS)
    per_core = _host_inputs(plan, x, W1, Wmu, Wls, bmu, bls, gamma, beta)

    geom = _geom(plan, CALL_COLS)
    if geom not in _CACHE:
        _CACHE[geom] = _build_program(geom)
    nc = _CACHE[geom]

    outs, _ = _run_hw(nc, per_core, trace=False)
    mu, ls = _postprocess(plan, outs)
    return mu, ls


# "M2" key is provided by _host_inputs; keep name mapping for clarity.


# revision 50
# speedup vs baseline: 1.0233x; 1.0081x over previous
"""GCN encoder (2x GCNConv + BatchNorm/ReLU) on 8 Trainium2 NeuronCores.

Math: with s = 1/sqrt(deg+1) (deg = in-degree by dst), the GCN edge norm
factorizes: norm_e = s[src]*s[dst], so for any node features H,
    A(H) := segsum(norm_e * H[src], dst) + H * s^2
          = s * ( segsum( (s*H)[src], dst) + (s*H) )
and GCNConv(H, W, b) = A(H)@W + b = A(H@W) + b, so the whole net needs only
TWO sparse aggregations (layer1 on (s*x)@W1, layer2 on s*post-BN hidden),
and mu / log_std share the second one.

Layer 1 (pull): by linearity A(x@W1) == A(x)@W1, the host-marshaled bf16 x
tables (replicated to every core's DRAM as inputs) ARE the gather tables --
no collective at all.  Slot-aligned gathers + identity-matmul PSUM
accumulation per dst block, then W1 applied post-aggregation.

Layer 2 (push + ReduceScatter; replaces the 253us tab2 AllGather):
  * After BN each core writes its local post-BN slab (s * relu(bn(h1)))
    to local DRAM (tab2loc) -- no collective.
  * Edges are partitioned by SRC core.  Each core gathers its outgoing
    edges' messages from tab2loc in chunks of 128 (dst-sorted), and the PE
    accumulates them FEATURE-MAJOR into per-(dst core, 4-block group) PSUM
    tiles via per-chunk 0/1 assignment matmuls:
        psumT[f, dst] += msg_chunk[e, f]^T  @  M_chunk[e, dst_window]
    (lhsT = the gathered chunk, rhs = a host-built 0/1 matrix).  PE sums
    are deterministic -- dma_scatter_add would race on duplicate dst rows
    on real hardware (verified experimentally).
  * Self-loop term rides along as synthetic (j -> j) edges.
  * Partial slabs go to a [8*128, npc] bf16 accumulator; ONE bf16
    ReduceScatter (priced by OUT size = 1.6MB -> ~57us vs 253us AllGather)
    hands each core its final aggregated feature-major slab.
  * Epilogue: scale by s (column-wise, feature-major), then lhsT IS already
    transposed for the Wcat matmul -- no per-block PE transposes needed.

Static SPMD choreography: chunk counts, matmul windows, and M offsets are
max-over-cores static geometry; all per-core variation lives in tensor
contents (gather idx, M values), with pad positions pointing at row 0 and
zero M columns.

Gather calls carry <=896 idxs (hw SWDGE descriptor ring holds 128 in-flight
entries per engine; larger calls hang the device).  Calls rotate over 4
SWDGE queues; queue_num is re-derived post-compile from the Tile-assigned
DMASW sem lane.
"""

import numpy as np

N_NODES = 50000
N_EDGES = 800000
D_IN = 128
D_HID = 128
D_LAT = 64
BN_EPS = 1e-5
N_CORES = 8
P = 128
LO_CORES = 5  # cores 0..4 form the "lo" table half; 5*6272=31360 < 32768
              # (dma_gather int16 indices address at most 32768 rows per call)

CALL_COLS = 7      # gather call size: 7 cols * 128 = 896 idxs (hw ring cap)
NUM_Q = 4          # SWDGE queues
GB = 4             # dst blocks per PSUM group (layer 2 push)

_CACHE = {}


# ----------------------------------------------------------------------------
# Host-side preprocessing
# ----------------------------------------------------------------------------


def _wrap_idx(lin):
    """dma_gather idx layout: position i -> [i%16, i//16], replicated to 128
    partitions. lin: [n] int array (n % 16 == 0) -> [128, n//16] int16."""
    n = lin.shape[0]
    w = lin.reshape(n // 16, 16).T.astype(np.int16)  # [16, n//16]
    return np.tile(w, (8, 1))


def _pack_calls(D, call_cols):
    """Slice the global column space into calls of <= call_cols columns.

    A call may cover partial blocks; each call carries its piece list
    [(block, col_off_in_call, width, first, last)].
    """
    C0 = np.concatenate([[0], np.cumsum(D)]).astype(np.int64)
    ct = int(C0[-1])
    calls = []
    for c0 in range(0, ct, call_cols):
        c1 = min(c0 + call_cols, ct)
        pieces = []
        for b in range(len(D)):
            lo = max(c0, int(C0[b]))
            hi = min(c1, int(C0[b + 1]))
            if lo < hi:
                pieces.append(
                    (b, lo - c0, hi - lo, lo == int(C0[b]), hi == int(C0[b + 1]))
                )
        calls.append((c0, c1 - c0, tuple(pieces)))
    return C0, calls


def _build_pass(tcoord_src, tkey_dst, n_cores, npc, blocks, call_cols,
                pad_idx, idx_base):
    """Build one gather pass layout (layer-1 pull).

    tcoord_src: per-edge source table coord (already offset for hi pass)
    tkey_dst:   per-edge dst node key in THIS pass's permutation
    Returns D [blocks], C0, calls, idx arrays [n_cores, 128, c_total] int32.
    """
    deg = np.bincount(tkey_dst, minlength=n_cores * npc)
    d3 = deg.reshape(n_cores, blocks, P)
    D = d3.max(axis=(0, 2)).astype(np.int64)
    D = np.maximum(D, 1)
    C0, calls = _pack_calls(D, call_cols)
    c_total = int(C0[-1])

    idx = np.full((n_cores, P, c_total), pad_idx - idx_base, dtype=np.int32)
    eorder = np.argsort(tkey_dst, kind="stable")
    k_s = tkey_dst[eorder]
    src_s = (tcoord_src[eorder] - idx_base).astype(np.int32)
    grp = np.searchsorted(k_s, k_s)
    slot = np.arange(k_s.size) - grp
    core_e = k_s // npc
    local_e = k_s % npc
    b_e = local_e // P
    p_e = local_e % P
    col_e = C0[b_e] + slot
    assert (slot < D[b_e]).all()
    idx[core_e, p_e, col_e] = src_s
    return D, C0, calls, idx, c_total


def _idx_to_wrapped(idx):
    """[n_cores, 128, c_total] int32 -> wrapped int16 [n_cores, 128, 8*c_total].

    Global linear position order is column-major (i = c*128 + p); contiguous
    position chunks map to contiguous wrapped columns, so any call covering
    cols [c0, c1) reads the wrapped slice [:, 8*c0 : 8*c1]."""
    n_cores, _, c_total = idx.shape
    out = np.empty((n_cores, 128, 8 * c_total), dtype=np.int16)
    for k in range(n_cores):
        lin = idx[k].T.reshape(-1)
        out[k] = _wrap_idx(lin)
    return out


def _plan_l2(src, dst, core_of, local_of, l2col_of, node_of, npc, blocks,
             s_l2):
    """Layer-2 push plan: per-src-core edge streams, static chunk/window
    choreography, per-core gather idx + 0/1 M matrices.

    The dst COLUMN order is a separate per-core permutation (l2col_of,
    sorted by total degree) -- balanced across src cores, unlike the main
    dlo-sorted layout, so the max-over-cores chunk envelope stays tight.

    Static geometry (identical across cores, SPMD):
      groups: (kk, g, g0, W, C, m_off, m_cols, windows=((w0, w1, mo), ...))
    Per-core data: idx stream (int, gather rows into tab2loc), M [128, Mtot].
    """
    n_cores = N_CORES
    GR = (blocks + GB - 1) // GB
    group_w = [min(GB * P, npc - g * GB * P) for g in range(GR)]

    # per-core edge lists sorted by (dst_core, dst_l2pos).  Self loops are
    # NOT included here -- they would land only on the own-core stream and
    # blow up the max-over-cores static envelope by ~n_local per dst core;
    # the self term is added in the epilogue from a transposed local gather
    # that runs during the ReduceScatter wait.
    ecore = []
    for k in range(n_cores):
        m = core_of[src] == k
        s_loc = local_of[src[m]]
        d_core = core_of[dst[m]]
        d_loc = l2col_of[dst[m]]
        key = d_core * npc + d_loc
        o = np.argsort(key, kind="stable")
        ecore.append((s_loc[o], d_core[o], d_loc[o], key[o]))

    # group slices per core: searchsorted bounds on key
    # chunk counts per (kk, g): ceil(max_core n / 128)
    groups = []
    per_core_chunks = [[] for _ in range(n_cores)]  # list of (idx128, jrel128)
    m_off = 0
    order_kg = [(kk, g) for kk in range(n_cores) for g in range(GR)]
    for kk, g in order_kg:
        if True:
            g0 = g * GB * P
            W = group_w[g]
            lo_key = kk * npc + g0
            hi_key = kk * npc + g0 + W
            segs = []
            for k in range(n_cores):
                keys = ecore[k][3]
                a = np.searchsorted(keys, lo_key)
                b = np.searchsorted(keys, hi_key)
                segs.append((a, b))
            nmax = max(b - a for a, b in segs)
            C = (nmax + P - 1) // P
            windows = []
            g_m0 = m_off
            for c in range(C):
                w0, w1 = W, 0
                for k in range(n_cores):
                    a, b = segs[k]
                    r0, r1 = a + c * P, min(a + (c + 1) * P, b)
                    if r0 < r1:
                        j = ecore[k][2][r0:r1] - g0
                        w0 = min(w0, int(j.min()))
                        w1 = max(w1, int(j.max()) + 1)
                if w1 <= w0:
                    w0, w1 = 0, 1
                windows.append((w0, w1, m_off - g_m0))
                m_off += w1 - w0
            groups.append((kk, g, g0, W, C, g_m0, m_off - g_m0,
                           tuple(windows)))
            for k in range(n_cores):
                a, b = segs[k]
                for c in range(C):
                    r0, r1 = a + c * P, min(a + (c + 1) * P, b)
                    n = max(0, r1 - r0)
                    idx128 = np.zeros(P, np.int32)
                    jrel = np.full(P, -1, np.int32)
                    if n > 0:
                        sl = ecore[k][0][r0:r1]
                        idx128[:n] = (sl % P) * blocks + sl // P
                        jrel[:n] = ecore[k][2][r0:r1] - g0
                    per_core_chunks[k].append((idx128, jrel))

    total_chunks = sum(gr[4] for gr in groups)
    m_total = m_off
    n_calls = (total_chunks + CALL_COLS - 1) // CALL_COLS

    # per-core tensors
    widx2 = []
    Ms = []
    for k in range(n_cores):
        lin = np.concatenate([c[0] for c in per_core_chunks[k]])
        widx2.append(_wrap_idx(lin))
        M = np.zeros((P, m_total), np.float32)
        t = 0
        for kk, g, g0, W, C, g_m0, g_mc, windows in groups:
            for c in range(C):
                idx128, jrel = per_core_chunks[k][t]
                w0, w1, mo = windows[c]
                rows = np.nonzero(jrel >= 0)[0]
                # M carries the dst-side s factor (out = s * sum), so the
                # epilogue needs no post-matmul scaling at all.
                np.add.at(M, (rows, g_m0 + mo + (jrel[rows] - w0)),
                          s_l2[kk][g0 + jrel[rows]])
                t += 1
        Ms.append(M)

    max_mw = max((gr[6] for gr in groups), default=1)
    max_win = max((w1 - w0 for gr in groups for (w0, w1, _) in gr[7]),
                  default=1)
    return dict(
        groups=tuple(groups),
        total_chunks=total_chunks,
        m_total=m_total,
        n_calls=n_calls,
        max_mw=max_mw,
        max_win=max_win,
        widx2=widx2,
        Ms=Ms,
    )


def _plan(edge_index, n_nodes, n_cores, call_cols):
    src = np.asarray(edge_index[0], dtype=np.int64)
    dst = np.asarray(edge_index[1], dtype=np.int64)

    deg_in = np.bincount(dst, minlength=n_nodes).astype(np.int64)
    s = (1.0 / np.sqrt((deg_in + 1).astype(np.float64))).astype(np.float32)

    n_local = (n_nodes + n_cores - 1) // n_cores
    blocks = (n_local + 1 + P - 1) // P
    npc = blocks * P
    lo_rows = LO_CORES * npc
    assert lo_rows < 32768 and (n_cores * npc - lo_rows) < 32768

    # ---- core assignment: deal by total-degree rank (balances edge load and
    # aligns block-degree profiles across cores).
    order = np.argsort(-deg_in, kind="stable")
    rank_of = np.empty(n_nodes, dtype=np.int64)
    rank_of[order] = np.arange(n_nodes)
    core_of = rank_of % n_cores

    src_is_lo = core_of[src] < LO_CORES
    dlo = np.bincount(dst[src_is_lo], minlength=n_nodes)
    dhi = np.bincount(dst[~src_is_lo], minlength=n_nodes)

    # main layout: per-core locals sorted by lo-degree (tight LO padding)
    local_of = np.empty(n_nodes, dtype=np.int64)
    node2hi = np.empty(n_nodes, dtype=np.int64)
    for k in range(n_cores):
        nodes_k = np.nonzero(core_of == k)[0]
        o = nodes_k[np.argsort(-dlo[nodes_k], kind="stable")]
        local_of[o] = np.arange(o.size)
        o2 = nodes_k[np.argsort(-dhi[nodes_k], kind="stable")]
        node2hi[o2] = k * npc + np.arange(o2.size)
    node2table = core_of * npc + local_of

    # ---- gather-source row numbering: tables are [(core,part), (block,feat)]
    # 2-D tensors, so node (core k, local j=b*128+p) lives at flat row
    # (k*128+p)*blocks + b of its half (hi half: k-LO_CORES).
    def kpb_row(core, local, core0):
        return ((core - core0) * P + local % P) * blocks + local // P

    node2row = np.where(
        core_of < LO_CORES,
        kpb_row(core_of, local_of, 0),
        kpb_row(core_of, local_of, LO_CORES),
    )
    pad_lo = kpb_row(0, npc - 1, 0)
    pad_hi = kpb_row(N_CORES - 1, npc - 1, LO_CORES)

    # ---- LO pass on the main permutation
    D_lo, C0_lo, calls_lo, idx_lo, ct_lo = _build_pass(
        node2row[src[src_is_lo]], node2table[dst[src_is_lo]], n_cores, npc,
        blocks, call_cols, pad_lo, 0,
    )

    # ---- HI pass on the hi permutation
    D_hi, C0_hi, calls_hi, idx_hi, ct_hi = _build_pass(
        node2row[src[~src_is_lo]], node2hi[dst[~src_is_lo]], n_cores, npc,
        blocks, call_cols, pad_hi, 0,
    )

    # ---- combine map: main-layout local j gets acc_hi[himap[j]] added
    himap = np.full((n_cores, npc), npc - 1, dtype=np.int64)  # pad -> pad row
    for k in range(n_cores):
        nodes_k = np.nonzero(core_of == k)[0]
        himap[k, local_of[nodes_k]] = node2hi[nodes_k] % npc

    widx_lo = _idx_to_wrapped(idx_lo)
    widx_hi = _idx_to_wrapped(idx_hi)
    himap_row = (himap % P) * blocks + himap // P
    widx_cb = np.stack([_wrap_idx(himap_row[k]) for k in range(n_cores)])

    # per-core node lists and s in the MAIN layout
    node_of = np.full((n_cores, npc), -1, dtype=np.int64)
    s_arr = np.zeros((n_cores, P, blocks), dtype=np.float32)
    for k in range(n_cores):
        nodes_k = np.nonzero(core_of == k)[0]
        loc = local_of[nodes_k]
        node_of[k, loc] = nodes_k
        s_arr[k, loc % P, loc // P] = s[nodes_k]

    # layer-2 dst column permutation: per-core total-degree sort (balanced
    # per-src-core edge counts -> tight static chunk envelope)
    l2col_of = np.empty(n_nodes, dtype=np.int64)
    node_of_l2 = np.full((n_cores, npc), -1, dtype=np.int64)
    s_l2 = np.zeros((n_cores, npc), dtype=np.float32)
    wself = []
    for k in range(n_cores):
        nodes_k = np.nonzero(core_of == k)[0]
        o = nodes_k[np.argsort(-deg_in[nodes_k], kind="stable")]
        l2col_of[o] = np.arange(o.size)
        node_of_l2[k, : o.size] = o
        s_l2[k, : o.size] = s[o]
        rows = np.zeros(npc, dtype=np.int64)
        loc = local_of[o]
        rows[: o.size] = (loc % P) * blocks + loc // P
        wself.append(_wrap_idx(rows))

    l2 = _plan_l2(src, dst, core_of, local_of, l2col_of, node_of, npc, blocks,
                  s_l2)

    return dict(
        s=s,
        node2table=node2table,
        node_of=node_of,
        npc=npc,
        blocks=blocks,
        n_local=n_local,
        lo_rows=lo_rows,
        s_arr=s_arr,
        D_lo=D_lo, C0_lo=C0_lo, calls_lo=calls_lo, idx_lo=idx_lo, ct_lo=ct_lo,
        D_hi=D_hi, C0_hi=C0_hi, calls_hi=calls_hi, idx_hi=idx_hi, ct_hi=ct_hi,
        himap=himap,
        widx_lo=widx_lo, widx_hi=widx_hi, widx_cb=widx_cb,
        node_of_l2=node_of_l2, s_l2=s_l2, wself=wself,
        l2=l2,
    )


def _to_bf16(a):
    import ml_dtypes

    return a.astype(ml_dtypes.bfloat16)


def _host_inputs(plan, x, W1, Wmu, Wls, bmu, bls, gamma, beta):
    npc = plan["npc"]
    node_of = plan["node_of"]
    s = plan["s"]
    l2 = plan["l2"]
    wcat = np.concatenate([Wmu, Wls], axis=1).astype(np.float32)
    bcat = np.concatenate([bmu, bls]).astype(np.float32).reshape(D_HID, 1)

    # s-prescaled x in the partition-major table layout:
    # row (core*128 + p), col (block*128 + f) holds node (core, b*128+p).
    blocks = npc // P
    xs_tab = np.zeros((N_CORES * npc, x.shape[1]), dtype=np.float32)
    for k in range(N_CORES):
        nodes = node_of[k]
        valid = nodes >= 0
        xs_tab[k * npc + np.nonzero(valid)[0]] = (
            x[nodes[valid]] * s[nodes[valid]][:, None]
        )
    xs_kpb = (
        xs_tab.reshape(N_CORES, blocks, P, D_IN)
        .transpose(0, 2, 1, 3)
        .reshape(N_CORES * P, blocks * D_IN)
    )
    xs_kpb = _to_bf16(xs_kpb)
    xs_lo = np.ascontiguousarray(xs_kpb[: LO_CORES * P])
    xs_hi = np.ascontiguousarray(xs_kpb[LO_CORES * P :])

    per_core = []
    for k in range(N_CORES):
        sa = plan["s_arr"][k]  # [P, blocks]
        # layer-2 epilogue scale: s per output partition (L2 column order:
        # block b, partition p <-> l2 position b*128+p)
        scol2 = np.ascontiguousarray(
            plan["s_l2"][k].reshape(blocks, P).T.astype(np.float32))
        per_core.append(
            {
                "xslo": xs_lo,
                "xshi": xs_hi,
                "xso": np.ascontiguousarray(xs_kpb[k * P : (k + 1) * P]),
                "W1": np.ascontiguousarray(_to_bf16(W1)),
                "Wcat": np.ascontiguousarray(_to_bf16(wcat)),
                "bcat": bcat,
                "s_arr": np.ascontiguousarray(sa),
                "widx_lo": np.ascontiguousarray(plan["widx_lo"][k]),
                "widx_hi": np.ascontiguousarray(plan["widx_hi"][k]),
                "widx_cb": np.ascontiguousarray(plan["widx_cb"][k]),
                "gamma": gamma.astype(np.float32).reshape(D_HID, 1),
                "beta": beta.astype(np.float32).reshape(D_HID, 1),
                "widx2": np.ascontiguousarray(l2["widx2"][k]),
                "M2": np.ascontiguousarray(_to_bf16(l2["Ms"][k])),
                "scol2": scol2,
                "wself": np.ascontiguousarray(plan["wself"][k]),
            }
        )
    return per_core


def _postprocess(plan, outs):
    n_nodes = int(plan["node_of"].max()) + 1
    mu = np.zeros((n_nodes, D_LAT), dtype=np.float32)
    ls = np.zeros((n_nodes, D_LAT), dtype=np.float32)
    node_of = plan["node_of_l2"]  # out_cat columns are in the L2 permutation
    npc = node_of.shape[1]
    for k in range(N_CORES):
        nodes = node_of[k]
        valid = nodes >= 0
        o = np.asarray(outs[k]).astype(np.float32)  # [128 outf, npc] f-major
        cols = valid.nonzero()[0]
        mu[nodes[valid]] = o[:D_LAT, cols].T
        ls[nodes[valid]] = o[D_LAT:, cols].T
    return mu, ls


# ----------------------------------------------------------------------------
# Device program
# ----------------------------------------------------------------------------


def _build_program(geom):
    from concourse import bacc, bass, mybir, tile
    from concourse.masks import make_identity

    (npc, blocks, D_lo, calls_lo, ct_lo, D_hi, calls_hi, ct_hi, n_real,
     call_cols, lo_rows, l2g) = geom
    (groups, total_chunks, m_total, n_calls, max_mw, max_win) = l2g
    D_lo, D_hi = list(D_lo), list(D_hi)
    f32 = mybir.dt.float32
    bf16 = mybir.dt.bfloat16
    i16 = mybir.dt.int16
    GR = (blocks + GB - 1) // GB

    nc = bacc.Bacc("TRN2", target_bir_lowering=False, debug=False,
                   num_devices=N_CORES, num_swdge_queues=NUM_Q)

    t_xslo = nc.dram_tensor("xslo", [LO_CORES * P, npc], bf16,
                            kind="ExternalInput")
    t_xshi = nc.dram_tensor("xshi", [(N_CORES - LO_CORES) * P, npc], bf16,
                            kind="ExternalInput")
    t_xso = nc.dram_tensor("xso", [P, npc], bf16, kind="ExternalInput")
    t_W1 = nc.dram_tensor("W1", [P, D_HID], bf16, kind="ExternalInput")
    t_Wcat = nc.dram_tensor("Wcat", [D_HID, P], bf16, kind="ExternalInput")
    t_bcat = nc.dram_tensor("bcat", [P, 1], f32, kind="ExternalInput")
    t_sarr = nc.dram_tensor("s_arr", [P, blocks], f32, kind="ExternalInput")
    t_wlo = nc.dram_tensor("widx_lo", [P, 8 * ct_lo], i16, kind="ExternalInput")
    t_whi = nc.dram_tensor("widx_hi", [P, 8 * ct_hi], i16, kind="ExternalInput")
    t_wcb = nc.dram_tensor("widx_cb", [P, npc // 16], i16, kind="ExternalInput")
    t_gamma = nc.dram_tensor("gamma", [D_HID, 1], f32, kind="ExternalInput")
    t_beta = nc.dram_tensor("beta", [D_HID, 1], f32, kind="ExternalInput")
    t_widx2 = nc.dram_tensor("widx2", [P, total_chunks * P // 16], i16,
                             kind="ExternalInput")
    t_M2 = nc.dram_tensor("M2", [P, m_total], bf16, kind="ExternalInput")
    t_scol2 = nc.dram_tensor("scol2", [P, blocks], f32, kind="ExternalInput")
    t_wself = nc.dram_tensor("wself", [P, npc // 16], i16,
                             kind="ExternalInput")
    t_out = nc.dram_tensor("out_cat", [P, npc], bf16, kind="ExternalOutput")

    tab2loc = nc.dram_tensor("tab2loc", [npc, P], bf16)
    acc1 = nc.dram_tensor("acc_hi1", [P, npc], bf16)
    accL2 = nc.dram_tensor("accL2", [N_CORES * P, npc], bf16)
    rs_out = nc.dram_tensor("rs_out", [P, npc], bf16)
    st_in = nc.dram_tensor("st_in", [2 * D_HID], f32)
    st_out = nc.dram_tensor("st_out", [N_CORES * 2 * D_HID], f32,
                            addr_space="Shared")

    groups_rep = [list(range(N_CORES))]
    inv_n = 1.0 / float(n_real)

    with tile.TileContext(nc) as tc:
        with (
            tc.tile_pool(name="persist", bufs=1) as persist,
            tc.tile_pool(name="stream", bufs=4) as stream,
            tc.tile_pool(name="gath", bufs=3) as gath,
            tc.tile_pool(name="gath2", bufs=4) as gath2,
            tc.tile_pool(name="mpool", bufs=3) as mpool,
            tc.tile_pool(name="ppool", bufs=3) as ppool,
            tc.tile_pool(name="gathcb", bufs=1) as gathcb,
            tc.tile_pool(name="small", bufs=1) as small,
            tc.tile_pool(name="ps", bufs=2, space="PSUM") as psp,
            tc.tile_pool(name="ps_blk", bufs=3, space="PSUM") as psblk,
            tc.tile_pool(name="ps_acc", bufs=1, space="PSUM") as psacc,
        ):
            H = persist.tile([P, npc], bf16, tag="H")
            ownslab = persist.tile([P, npc], bf16, tag="ownslab")
            accsb = persist.tile([P, npc], bf16, tag="accsb")
            outsb = persist.tile([P, npc], bf16, tag="outsb")
            wlo = persist.tile([P, 8 * ct_lo], i16, tag="wlo")
            whi = persist.tile([P, 8 * ct_hi], i16, tag="whi")
            wcb = persist.tile([P, npc // 16], i16, tag="wcb")
            widx2 = persist.tile([P, total_chunks * P // 16], i16,
                                 tag="widx2")
            wself = persist.tile([P, npc // 16], i16, tag="wself")
            selfT = persist.tile([P, npc], bf16, tag="selfT")
            w1 = small.tile([P, D_HID], bf16, tag="w1")
            wcat = small.tile([D_HID, P], bf16, tag="wcat")
            sarr = small.tile([P, blocks], f32, tag="sarr")
            scol2 = small.tile([P, blocks], f32, tag="scol2")
            gcol = small.tile([D_HID, 1], f32, tag="gcol")
            bcol = small.tile([D_HID, 1], f32, tag="bcol")
            bccol = small.tile([P, 1], f32, tag="bccol")
            ident = small.tile([P, P], bf16, tag="ident")
            identf = small.tile([P, P], f32, tag="identf")
            zeroM = small.tile([P, GB * P], bf16, tag="zeroM")
            ones_col = small.tile([P, 1], bf16, tag="ones_col")
            ones_row = small.tile([1, P], f32, tag="ones_row")
            grep = small.tile([P, P], f32, tag="grep")
            brep = small.tile([P, P], f32, tag="brep")
            grep4 = small.tile([P, 4 * P], bf16, tag="grep4")
            brep4 = small.tile([P, 4 * P], bf16, tag="brep4")
            bcrep = small.tile([P, P], f32, tag="bcrep")
            bcrep4 = small.tile([P, 4 * P], f32, tag="bcrep4")

            # split the idx loads so the first gather calls start early
            h8 = min(8 * 8 * CALL_COLS, 8 * ct_hi)
            nc.sync.dma_start(out=whi[:, :h8], in_=t_whi[:, :h8])
            nc.sync.dma_start(out=whi[:, h8:], in_=t_whi[:, h8:])
            l8 = min(8 * 8 * CALL_COLS, 8 * ct_lo)
            nc.sync.dma_start(out=wlo[:, :l8], in_=t_wlo[:, :l8])
            nc.sync.dma_start(out=wlo[:, l8:], in_=t_wlo[:, l8:])
            nc.sync.dma_start(out=wcb[:], in_=t_wcb[:])
            nc.sync.dma_start(out=widx2[:], in_=t_widx2[:])
            nc.sync.dma_start(out=wself[:], in_=t_wself[:])
            nc.sync.dma_start(out=scol2[:], in_=t_scol2[:])
            nc.sync.dma_start(out=w1[:], in_=t_W1[:])
            nc.sync.dma_start(out=wcat[:], in_=t_Wcat[:])
            nc.sync.dma_start(out=sarr[:], in_=t_sarr[:])
            nc.sync.dma_start(out=gcol[:], in_=t_gamma[:])
            nc.sync.dma_start(out=bcol[:], in_=t_beta[:])
            nc.sync.dma_start(out=bccol[:], in_=t_bcat[:])
            make_identity(nc, ident[:])
            make_identity(nc, identf[:])
            nc.vector.memset(zeroM[:], 0.0)
            nc.vector.memset(ones_col[:], 1.0)
            nc.vector.memset(ones_row[:], 1.0)
            warm = small.tile([1, 1], f32, tag="warm")
            nc.vector.memset(warm[:], 1.0)
            nc.scalar.activation(out=warm[:], in_=warm[:],
                                 func=mybir.ActivationFunctionType.Sqrt)

            def outer_bcast(col_ap, dst_tile):
                pst = psp.tile([P, P], f32, space="PSUM", tag="ps_big")
                nc.tensor.transpose(out=pst[0:1, :], in_=col_ap,
                                    identity=identf[:])
                row = stream.tile([1, P], f32, tag="rowbuf")
                nc.vector.tensor_copy(out=row[:], in_=pst[0:1, :])
                psb = psp.tile([P, P], f32, space="PSUM", tag="ps_big")
                nc.tensor.matmul(out=psb[:], lhsT=ones_row[:], rhs=row[:],
                                 start=True, stop=True)
                nc.vector.tensor_copy(out=dst_tile[:], in_=psb[:])

            outer_bcast(bccol[:], bcrep)
            for r in range(4):
                nc.vector.tensor_copy(out=bcrep4[:, r * P : (r + 1) * P],
                                      in_=bcrep[:])

            # --- layer 1 gathers raw (s*x): A(x@W1) == A(x)@W1, so W1 is
            # applied per-block AFTER aggregation; the host-marshaled xslo/
            # xshi inputs ARE the gather tables (no stage-1, no table write).
            nc.sync.dma_start(out=ownslab[:], in_=t_xso[:, :])

            ps_stats = psacc.tile([D_HID, 2], f32, space="PSUM",
                                  tag="ps_stats")

            qn = [0]

            def next_q():
                q = qn[0]
                qn[0] = (qn[0] + 1) % NUM_Q
                return q

            def gather(out_ap, table_ap, widx_ap, n_idx):
                nc.gpsimd.dma_gather(
                    out_ap.rearrange("p (c f) -> p c f", f=P),
                    table_ap,
                    widx_ap,
                    num_idxs=n_idx,
                    num_idxs_reg=n_idx,
                    elem_size=P,
                    queue_num=next_q(),
                )

            def aggregate(tlo, thi, acc_t, selfslab):
                """Layer-1 pull aggregation -> H blocks + BN stats."""
                grp = {}

                def blk_view(b, pre):
                    g, r = b // GB, b % GB
                    if g not in grp:
                        grp[g] = psblk.tile([P, GB * P], f32, space="PSUM",
                                            tag="ps_blk",
                                            name=f"{pre}_{g}")
                    return grp[g][:, r * P : (r + 1) * P]

                g_open = set()

                def g_last(b):
                    return min((b // GB) * GB + GB - 1, blocks - 1)

                def flags(b, is_first_mm, is_last_mm):
                    g = b // GB
                    start = is_first_mm and g not in g_open
                    if start:
                        g_open.add(g)
                    stop = is_last_mm and b == g_last(b)
                    return start, stop

                for c0, cols, pieces in calls_hi:
                    gt = gath.tile([P, call_cols * P], bf16, tag="gt")
                    gather(gt[:, : cols * P],
                           thi.rearrange("q (r f) -> (q r) f", f=P),
                           whi[:, 8 * c0 : 8 * (c0 + cols)], cols * P)
                    for b, o, d, first, last in pieces:
                        bv = blk_view(b, "psg_hi")
                        for i in range(d):
                            st_, sp_ = flags(b, first and i == 0,
                                             last and i == d - 1)
                            nc.tensor.matmul(
                                out=bv, lhsT=ident[:],
                                rhs=gt[:, (o + i) * P : (o + i + 1) * P],
                                start=st_, stop=sp_)
                        if last and b == g_last(b):
                            g = b // GB
                            w = (b % GB) + 1
                            nc.vector.tensor_copy(
                                out=accsb[:, g * GB * P : g * GB * P + w * P],
                                in_=grp.pop(g)[:, : w * P])
                nc.sync.dma_start(out=acc_t[:, :], in_=accsb[:])

                # combine gathers (cbt[:, j] = acc_t[himap[j]]) are issued
                # a few calls INTO the lo stream so the acc-write latency
                # hides behind lo gather work.
                cbt = gathcb.tile([P, npc], bf16, tag="gtcb")

                grp.clear()
                g_open.clear()

                def start_block(b):
                    bv = blk_view(b, "psg_lo")
                    st_, _ = flags(b, True, False)
                    nc.tensor.matmul(out=bv, lhsT=ident[:],
                                     rhs=selfslab[:, b * P : (b + 1) * P],
                                     start=st_, stop=False)
                    return bv

                # late enough that the acc_hi write (~5us) has landed by the
                # time the cb gather hits the Pool queue (SEQ-held waits
                # would stall the lo stream), early enough for block 0's
                # combine matmul.
                cb_at = min(max(4, 1), max((D_lo[0] - 1) // call_cols, 1))
                for ci, (c0, cols, pieces) in enumerate(calls_lo):
                    if ci == cb_at:
                        for cb0 in range(0, blocks, call_cols):
                            cb1 = min(cb0 + call_cols, blocks)
                            gather(cbt[:, cb0 * P : cb1 * P],
                                   acc_t.rearrange("q (r f) -> (q r) f", f=P),
                                   wcb[:, 8 * cb0 : 8 * cb1],
                                   (cb1 - cb0) * P)
                    gt = gath.tile([P, call_cols * P], bf16, tag="gt")
                    gather(gt[:, : cols * P],
                           tlo.rearrange("q (r f) -> (q r) f", f=P),
                           wlo[:, 8 * c0 : 8 * (c0 + cols)], cols * P)
                    for b, o, d, first, last in pieces:
                        if first:
                            start_block(b)
                        bv = blk_view(b, "psg_lo")
                        for i in range(d):
                            nc.tensor.matmul(
                                out=bv, lhsT=ident[:],
                                rhs=gt[:, (o + i) * P : (o + i + 1) * P],
                                start=False, stop=False)
                        if last:
                            _, sp_ = flags(b, False, True)
                            nc.tensor.matmul(
                                out=bv, lhsT=ident[:],
                                rhs=cbt[:, b * P : (b + 1) * P],
                                start=False, stop=sp_)
                        if last and b == g_last(b):
                            g = b // GB
                            gt_ps = grp.pop(g)
                            for r in range(b % GB + 1):
                                bb = g * GB + r
                                sl = slice(bb * P, (bb + 1) * P)
                                # U = s*(agg); h1 = U @ W1
                                ub = stream.tile([P, P], bf16, tag="ub")
                                nc.vector.tensor_scalar_mul(
                                    ub[:], gt_ps[:, r * P : (r + 1) * P],
                                    sarr[:, bb : bb + 1])
                                psT = psp.tile([P, P], bf16, space="PSUM",
                                               tag="ps_bigT")
                                nc.tensor.transpose(out=psT[:], in_=ub[:],
                                                    identity=ident[:])
                                uT = stream.tile([P, P], bf16, tag="uT")
                                nc.vector.tensor_copy(out=uT[:], in_=psT[:])
                                ps1 = psp.tile([P, P], f32, space="PSUM",
                                               tag="ps_big")
                                nc.tensor.matmul(out=ps1[:], lhsT=uT[:],
                                                 rhs=w1[:], start=True,
                                                 stop=True)
                                nc.vector.tensor_copy(out=H[:, sl],
                                                      in_=ps1[:])
                                sq = stream.tile([P, D_HID], bf16,
                                                 tag="sq")
                                nc.scalar.square(out=sq[:], in_=H[:, sl])
                                nc.tensor.matmul(
                                    out=ps_stats[:, 0:1], lhsT=H[:, sl],
                                    rhs=ones_col[:],
                                    start=(bb == 0), stop=False)
                                nc.tensor.matmul(
                                    out=ps_stats[:, 1:2], lhsT=sq[:],
                                    rhs=ones_col[:],
                                    start=False,
                                    stop=(bb == blocks - 1))

            aggregate(t_xslo, t_xshi, acc1, ownslab)

            # --- BN ---------------------------------------------------------
            st = small.tile([D_HID, 2], f32, tag="st")
            nc.vector.tensor_copy(out=st[:], in_=ps_stats[:])
            nc.sync.dma_start(out=st_in[:], in_=st[:])
            nc.gpsimd.collective_compute(
                "AllGather", mybir.AluOpType.bypass,
                replica_groups=groups_rep,
                ins=[st_in[:]], outs=[st_out[:]],
            )
            st8 = small.tile([D_HID, 2 * N_CORES], f32, tag="st8")
            nc.sync.dma_start(
                out=st8.rearrange("p (c t) -> p c t", t=2),
                in_=st_out[:].rearrange("(c p t) -> p c t", p=D_HID, t=2),
            )
            st2 = small.tile([D_HID, 2], f32, tag="st2")
            nc.vector.reduce_sum(
                out=st2[:],
                in_=st8.rearrange("p (c t) -> p t c", t=2),
                axis=mybir.AxisListType.X)

            eps_col = small.tile([D_HID, 1], f32, tag="eps_col")
            nc.vector.memset(eps_col[:], BN_EPS)
            mean = small.tile([D_HID, 1], f32, tag="mean")
            msq = small.tile([D_HID, 1], f32, tag="msq")
            var = small.tile([D_HID, 1], f32, tag="var")
            std = small.tile([D_HID, 1], f32, tag="std")
            istd = small.tile([D_HID, 1], f32, tag="istd")
            gp = small.tile([D_HID, 1], f32, tag="gp")
            bp_ = small.tile([D_HID, 1], f32, tag="bp")
            nc.vector.tensor_scalar_mul(mean[:], st2[:, 0:1], inv_n)
            nc.vector.tensor_scalar_mul(msq[:], st2[:, 1:2], inv_n)
            nc.scalar.square(out=var[:], in_=mean[:])
            nc.vector.tensor_tensor(out=var[:], in0=msq[:], in1=var[:],
                                    op=mybir.AluOpType.subtract)
            nc.scalar.activation(out=std[:], in_=var[:],
                                 func=mybir.ActivationFunctionType.Sqrt,
                                 bias=eps_col[:])
            nc.vector.reciprocal(out=istd[:], in_=std[:])
            nc.vector.tensor_tensor(out=gp[:], in0=gcol[:], in1=istd[:],
                                    op=mybir.AluOpType.mult)
            nc.vector.tensor_tensor(out=bp_[:], in0=mean[:], in1=gp[:],
                                    op=mybir.AluOpType.mult)
            nc.vector.tensor_tensor(out=bp_[:], in0=bcol[:], in1=bp_[:],
                                    op=mybir.AluOpType.subtract)
            outer_bcast(gp[:], grep)
            outer_bcast(bp_[:], brep)
            for r in range(4):
                nc.vector.tensor_copy(out=grep4[:, r * P : (r + 1) * P],
                                      in_=grep[:])
                nc.vector.tensor_copy(out=brep4[:, r * P : (r + 1) * P],
                                      in_=brep[:])

            # BN affine + ReLU + s-scale, written to local DRAM table
            # (tab2loc rows = p*blocks + b -- the layer-2 gather source).
            t2v = tab2loc[:, :].rearrange("(p b) f -> p b f", b=blocks)
            for b0 in range(0, blocks, 4):
                b1 = min(b0 + 4, blocks)
                w = (b1 - b0) * P
                t1 = stream.tile([P, 4 * P], bf16, tag="bn1")
                nc.vector.tensor_tensor(out=t1[:, :w], in0=H[:, b0 * P:b0 * P + w],
                                        in1=grep4[:, :w],
                                        op=mybir.AluOpType.mult)
                nc.vector.tensor_tensor(out=t1[:, :w], in0=t1[:, :w],
                                        in1=brep4[:, :w],
                                        op=mybir.AluOpType.add)
                nc.scalar.activation(out=t1[:, :w], in_=t1[:, :w],
                                     func=mybir.ActivationFunctionType.Relu)
                t2 = stream.tile([P, 4 * P], bf16, tag="bn2")
                for b in range(b0, b1):
                    r = b - b0
                    nc.vector.tensor_scalar_mul(
                        t2[:, r * P : (r + 1) * P],
                        t1[:, r * P : (r + 1) * P], sarr[:, b : b + 1])
                nc.sync.dma_start(out=t2v[:, b0:b1, :],
                                  in_=t2[:, :w].rearrange(
                                      "p (b f) -> p b f", f=P))

            # --- layer 2: push aggregation (feature-major M-matmuls) -------
            chunk_t = 0
            issued = {}

            def ensure_call(t):
                ci = t // CALL_COLS
                if ci not in issued:
                    t0 = ci * CALL_COLS
                    cols = min(CALL_COLS, total_chunks - t0)
                    gt = gath2.tile([P, CALL_COLS * P], bf16, tag="gt2")
                    gather(gt[:, : cols * P], tab2loc[:, :],
                           widx2[:, 8 * t0 : 8 * (t0 + cols)], cols * P)
                    issued[ci] = gt
                return issued[ci], t % CALL_COLS

            def emit_self():
                # self term: gather own rows in L2 column order, scale by s
                # (per-partition, node-major), transpose to feature-major.
                # Runs while the ReduceScatter holds the Pool queue.
                for c0 in range(0, blocks, CALL_COLS):
                    c1 = min(c0 + CALL_COLS, blocks)
                    sg = stream.tile([P, CALL_COLS * P], bf16, tag="sg")
                    gather(sg[:, : (c1 - c0) * P], tab2loc[:, :],
                           wself[:, 8 * c0 : 8 * c1], (c1 - c0) * P)
                    for b in range(c0, c1):
                        sgs = stream.tile([P, P], bf16, tag="sgs")
                        nc.vector.tensor_scalar_mul(
                            sgs[:], sg[:, (b - c0) * P : (b - c0 + 1) * P],
                            scol2[:, b : b + 1])
                        psT2 = psp.tile([P, P], bf16, space="PSUM",
                                        tag="ps_bigT")
                        nc.tensor.transpose(out=psT2[:], in_=sgs[:],
                                            identity=ident[:])
                        nc.vector.tensor_copy(
                            out=selfT[:, b * P : (b + 1) * P], in_=psT2[:])

            for gi, (kk, g, g0, W, C, g_m0, g_mc, windows) in enumerate(groups):
                Mt = None
                if g_mc > 0:
                    Mt = mpool.tile([P, max_mw], bf16, tag="mt")
                    nc.scalar.dma_start(out=Mt[:, :g_mc],
                                        in_=t_M2[:, g_m0 : g_m0 + g_mc])
                ps2 = psblk.tile([P, GB * P], f32, space="PSUM", tag="ps_blk",
                                 name=f"ps2_{gi}")
                nc.tensor.matmul(out=ps2[:, :W], lhsT=ident[:],
                                 rhs=zeroM[:, :W], start=True, stop=(C == 0))
                for c in range(C):
                    gt, pos = ensure_call(chunk_t)
                    w0, w1, mo = windows[c]
                    nc.tensor.matmul(
                        out=ps2[:, w0:w1],
                        lhsT=gt[:, pos * P : (pos + 1) * P],
                        rhs=Mt[:, mo : mo + (w1 - w0)],
                        start=False, stop=(c == C - 1))
                    chunk_t += 1
                pt = ppool.tile([P, GB * P], bf16, tag="pt")
                nc.vector.tensor_copy(out=pt[:, :W], in_=ps2[:, :W])
                nc.sync.dma_start(out=accL2[kk * P : (kk + 1) * P,
                                            g0 : g0 + W],
                                  in_=pt[:, :W])
            emit_self()

            nc.gpsimd.collective_compute(
                "ReduceScatter", mybir.AluOpType.add,
                replica_groups=groups_rep,
                ins=[accL2[:]], outs=[rs_out[:]],
            )

            # --- epilogue: out[outf, j] = wcat^T @ (rs + selfT) + bcat.
            # wcat is the STATIONARY operand (loaded once, no Ld churn); the
            # output is feature-major ([128 outf, npc]) and _postprocess
            # transposes on the host.  No post-scale (M/selfT carry s); bias
            # is a per-partition DVE scalar-add.  H loads ride the Act queue
            # so they don't serialize with the out stores on SP.
            def epilogue(rs_t, b0, b1, off):
                for c0 in range(b0, b1, GB):
                    c1 = min(c0 + GB, b1)
                    w = (c1 - c0) * P
                    nc.scalar.dma_start(
                        out=H[:, c0 * P : c0 * P + w],
                        in_=rs_t[:, c0 * P - off : c0 * P - off + w])
                    pse = psblk.tile([P, GB * P], f32, space="PSUM",
                                     tag="ps_blk", name=f"pse_{c0}")
                    nc.tensor.matmul(out=pse[:, :w], lhsT=wcat[:],
                                     rhs=H[:, c0 * P : c0 * P + w],
                                     start=True, stop=False)
                    nc.tensor.matmul(out=pse[:, :w], lhsT=wcat[:],
                                     rhs=selfT[:, c0 * P : c0 * P + w],
                                     start=False, stop=True)
                    nc.vector.tensor_scalar_add(
                        outsb[:, c0 * P : c0 * P + w], pse[:, :w], bccol[:])
                    nc.sync.dma_start(out=t_out[:, c0 * P : c0 * P + w],
                                      in_=outsb[:, c0 * P : c0 * P + w])

            epilogue(rs_out, 0, blocks, 0)

    nc.compile()
    if NUM_Q > 1:
        # Tile assigns SWDGE completion-sem lanes (DMASW0..7) round-robin in
        # SCHEDULED order, which differs from creation order.  Each sem lane
        # must be driven by a single SWDGE queue, so re-derive queue_num from
        # the assigned lane.
        import re as _re

        for _blk in nc.m.functions[0].blocks:
            for _inst in _blk.instructions:
                if type(_inst).__name__ == "InstDMAGatherAnt":
                    _m = _re.search(r"DMASW(\d+)_",
                                    str(_inst.sync_info.on_update[0]))
                    _inst.queue_num = int(_m.group(1)) % NUM_Q
    return nc


# ----------------------------------------------------------------------------
# Entry point
# ----------------------------------------------------------------------------

_IN_NAMES = ["xslo", "xshi", "xso", "W1", "Wcat", "bcat", "s_arr",
             "widx_lo", "widx_hi", "widx_cb", "gamma", "beta",
             "widx2", "M2", "scol2", "wself"]


def _geom(plan, call_cols):
    l2 = plan["l2"]
    l2g = (
        tuple(l2["groups"]),
        int(l2["total_chunks"]),
        int(l2["m_total"]),
        int(l2["n_calls"]),
        int(l2["max_mw"]),
        int(l2["max_win"]),
    )
    return (
        plan["npc"],
        plan["blocks"],
        tuple(int(d) for d in plan["D_lo"]),
        tuple(plan["calls_lo"]),
        plan["ct_lo"],
        tuple(int(d) for d in plan["D_hi"]),
        tuple(plan["calls_hi"]),
        plan["ct_hi"],
        int(plan["node_of"].max()) + 1,
        call_cols,
        plan["lo_rows"],
        l2g,
    )


def _run_hw(nc, per_core, trace=False, trace_cores=None):
    from concourse import bass_utils

    in_maps = [{nm: per_core[k][nm] for nm in _IN_NAMES} for k in range(N_CORES)]
    res = bass_utils.run_bass_kernel_spmd(
        nc, in_maps, core_ids=list(range(N_CORES)), trace=trace,
        trace_cores=trace_cores,
    )
    outs = [res.results[k]["out_cat"] for k in range(N_CORES)]
    return outs, res


def kernel(x, edge_index, W1, b1, gamma, beta, Wmu, bmu, Wls, bls):
    x = np.asarray(x, dtype=np.float32)
    edge_index = np.asarray(edge_index)
    W1 = np.asarray(W1, dtype=np.float32)
    gamma = np.asarray(gamma, dtype=np.float32)
    beta = np.asarray(beta, dtype=np.float32)
    Wmu = np.asarray(Wmu, dtype=np.float32)
    bmu = np.asarray(bmu, dtype=np.float32)
    Wls = np.asarray(Wls, dtype=np.float32)
    bls = np.asarray(bls, dtype=np.float32)

    plan = _plan(edge_index, x.shape[0], N_CORES, call_cols=CALL_COLS)
    per_core = _host_inputs(plan, x, W1, Wmu, Wls, bmu, bls, gamma, beta)

    geom = _geom(plan, CALL_COLS)
    if geom not in _CACHE:
        _CACHE[geom] = _build_program(geom)
    nc = _CACHE[geom]

    outs, _ = _run_hw(nc, per_core, trace=False)
    mu, ls = _postprocess(plan, outs)
    return mu, ls


# "M2" key is provided by _host_inputs; keep name mapping for clarity.
